# revision 39
# baseline (speedup 1.0000x reference)
"""MoE (BailingMoeV2.5) Trainium2 kernel — 8-core expert-parallel, SPARSE.

T=2048 tokens, H=2048 hidden, E=16 experts (4 groups, top-2 groups,
top-4 experts), I=1024 expert intermediate, shared expert IS=1024,
routed scale 2.5.

Each core owns 2 experts (host pairs high-count with low-count experts;
gather slot capacities 768/640, compute capacities trimmed to the
actual max routed counts + margin, rounded to 8):
  1. Router: logits via lossless-ish bf16 hi/lo split (3 bf16 passes),
     sigmoid scores, batched grouped top-k epilogue (3 pieces: 8/4/4
     token-tiles, overlapped with the score stream) -> per-token
     combine weights C2[token, 2] for this core's experts (x2.5,
     renormalized).
  2. Device-side stream compaction per expert (cumsum-matmul rank +
     batched one-hot mask build via big tensor_tensor ops + fused
     onehot matmuls) -> token-id list (int16, dma_gather layout) +
     per-slot weights.  Padding slots gather token 0 with W=0.
  3. dma_gather (transpose mode) pulls selected tokens from the bf16
     token-major x into feature-major [128, 16, cap].
  4. bf16 SwiGLU FFN per expert over the COMPUTE capacity only;
     output scaled by W -> z + ids exported.
  5. Shared expert (bf16) on the core's 256-token slice; shared-A
     blocks 0-4 interleave with router chunks (silu computed as
     x*sigmoid(x) to avoid ACT-table thrash), blocks 5-7 + shared
     pass C fill the compaction window.
Host unshard: out[ids] += z per (core, slot); out[slice_c] += shared_c.

DMA queues: sync(SP) = xhi router chunks then routed weights;
scalar(Act) = chunk-0 split + shared-expert feeds then outputs;
gpsimd = xlo router chunks, idx bookkeeping + gathers.
All large streams use host-side partition-contiguous layouts
(one >=4KB contiguous run per partition per DMA).
"""
import sys
from contextlib import ExitStack

sys.path.insert(0, "/opt/trn_rl_repo")

import numpy as np
import ml_dtypes

import concourse.bass as bass
import concourse.mybir as mybir
import concourse.tile as tile
from concourse import bacc
from concourse.bass_utils import run_bass_kernel_spmd
from concourse.masks import make_identity, make_upper_triangular

P = 128
T, H, E, K_TOP, I = 2048, 2048, 16, 4, 1024
G = 4
IS = 1024
N_CORES = 8
E_PER_CORE = E // N_CORES  # 2
TS = T // N_CORES          # 256
ROUTED_SCALE = 2.5

KT_H = H // P              # 16
KT_I = I // P              # 8
NTOK = 8                   # router token chunks of 256
TCH = T // NTOK            # 256
TT = T // P                # 16
CAPS = (768, 640)          # gather slot capacity (multiple of 128)
NC16 = 48                  # idx cols allocated (CAPS[0]/16)
ACH = 384                  # pass-A slot chunk (psum bank fits 384 fp32)
N_SA_ROUTER = 6            # shared-A blocks interleaved with router

F32 = mybir.dt.float32
F16 = mybir.dt.float16
BF16 = mybir.dt.bfloat16
I16 = mybir.dt.int16
I32 = mybir.dt.int32
AX = mybir.AxisListType.X
ALU = mybir.AluOpType
AF = mybir.ActivationFunctionType


def _halves(cap):
    return (ACH, cap - ACH)


def build_nc(capc):
    """capc: per-slot COMPUTE capacity (<= CAPS, multiple of 8)."""
    nc = bacc.Bacc(None, target_bir_lowering=False, debug=False)

    # all streaming inputs are partition-contiguous: [.., P, inner]
    xhi_d = nc.declare_dram_parameter("xhi", [NTOK, P, KT_H * TCH], BF16, isOutput=False)
    xlo_d = nc.declare_dram_parameter("xlo", [NTOK, P, KT_H * TCH], BF16, isOutput=False)
    xbf_d = nc.declare_dram_parameter("xbf", [T, H], BF16, isOutput=False)
    g_d = nc.declare_dram_parameter("g", [P, KT_H * 3 * E], BF16, isOutput=False)
    biasb_d = nc.declare_dram_parameter("biasb", [P, E], F32, isOutput=False)
    esel_d = nc.declare_dram_parameter("esel", [P, 2, E], F32, isOutput=False)
    w13_d = nc.declare_dram_parameter("w13", [E_PER_CORE, 2, 2, P, KT_H * 512], BF16, isOutput=False)
    w2_d = nc.declare_dram_parameter("w2", [E_PER_CORE, 4, P, KT_I * 512], BF16, isOutput=False)
    sw13_d = nc.declare_dram_parameter("sw13", [2, 4, P, KT_H * 256], BF16, isOutput=False)
    sw2_d = nc.declare_dram_parameter("sw2", [4, P, KT_I * 512], BF16, isOutput=False)
    xbs_d = nc.declare_dram_parameter("xbs", [P, KT_H * TS], BF16, isOutput=False)

    z_d = nc.declare_dram_parameter("z", [E_PER_CORE, CAPS[0], H], BF16, isOutput=True)
    ids_d = nc.declare_dram_parameter("ids", [E_PER_CORE, 16, NC16], I16, isOutput=True)
    out_d = nc.declare_dram_parameter("out", [TS, H], BF16, isOutput=True)

    def _c3(ap, n):
        return ap.rearrange("p (kt n) -> p kt n", n=n)

    with tile.TileContext(nc) as tc:
        with tc.tile_pool(name="res", bufs=1) as res:
            # ---------------- persistent small tiles ----------------
            sc_all = res.tile([P, TT, E], F32, name="sc_all")
            C2_sb = res.tile([P, TT, E_PER_CORE], F32, name="C2_sb")
            M2_sb = res.tile([P, TT, E_PER_CORE], F32, name="M2_sb")
            ident = res.tile([P, P], F32, name="ident")
            make_identity(nc, ident)
            tril = res.tile([P, P], F32, name="tril")
            make_upper_triangular(nc, tril, val=1.0, diag=True)
            ones128p = res.tile([P, 1], F32, name="ones128p")
            nc.vector.memset(ones128p, 1.0)
            ones_row = res.tile([1, P], F32, name="ones_row")
            nc.vector.memset(ones_row, 1.0)
            iotas = res.tile([P, 80], F32, name="iotas")
            iota16 = iotas[:, 0:16]
            iota48 = iotas[:, 16:64]
            tokid = iotas[:, 64:80]
            ii = res.tile([P, NC16], I32, name="ii")
            nc.gpsimd.iota(ii[:, 0:16], pattern=[[1, 16]], base=0, channel_multiplier=0)
            nc.vector.tensor_copy(iota16, ii[:, 0:16])
            nc.gpsimd.iota(ii[:, 0:NC16], pattern=[[1, NC16]], base=0, channel_multiplier=0)
            nc.vector.tensor_copy(iota48, ii[:, 0:NC16])
            nc.gpsimd.iota(ii[:, 0:TT], pattern=[[P, TT]], base=0, channel_multiplier=1)
            nc.vector.tensor_copy(tokid, ii[:, 0:TT])

            idx16 = [res.tile([P, NC16], I16, name=f"idx16_{k}")
                     for k in range(E_PER_CORE)]
            W128 = [res.tile([P, 6], F32, name=f"W128_{k}")
                    for k in range(E_PER_CORE)]
            W16 = [res.tile([16, NC16], F32, name=f"W16_{k}")
                   for k in range(E_PER_CORE)]
            # iota/token-id repeats for the batched compaction masks,
            # built up-front while the engines are otherwise idle
            i16r = res.tile([P, TT, 16], F16, name="i16r")
            nc.vector.tensor_copy(
                i16r, iota16[:, None, :].broadcast_to([P, TT, 16]))
            i48r = res.tile([P, TT, NC16], F16, name="i48r")
            nc.vector.tensor_copy(
                i48r, iota48[:, None, :].broadcast_to([P, TT, NC16]))
            tokr = res.tile([P, TT, NC16], F16, name="tokr")
            nc.vector.tensor_copy(
                tokr, tokid[:, :, None].broadcast_to([P, TT, NC16]))
            # block-identity BI[q, p] = (p % 16 == q), for idx broadcast
            BI = res.tile([16, P], F32, name="BI")
            bii = res.tile([16, P], I32, name="bii")
            nc.gpsimd.iota(bii, pattern=[[1, P]], base=0, channel_multiplier=0)
            nc.vector.tensor_scalar(bii, bii, 15, None, ALU.bitwise_and)
            bif = res.tile([16, P], F32, name="bif")
            nc.vector.tensor_copy(bif, bii)
            qcolf = res.tile([16, 1], F32, name="qcolf")
            qcol = res.tile([16, 1], I32, name="qcol")
            nc.gpsimd.iota(qcol, pattern=[[1, 1]], base=0, channel_multiplier=1)
            nc.vector.tensor_copy(qcolf, qcol)
            nc.vector.tensor_scalar(BI, bif, qcolf, None, ALU.is_equal)

            # shared-expert pools at outer scope
            es_ = ExitStack()
            swp = es_.enter_context(tc.tile_pool(name="sw", bufs=3))
            sres = es_.enter_context(tc.tile_pool(name="sres", bufs=1))
            so = es_.enter_context(tc.tile_pool(name="so", bufs=2))
            aps = es_.enter_context(tc.tile_pool(name="aps", bufs=1, space="PSUM"))
            # scalar (Act) HWDGE queue: chunk-0 xhi halves first (the
            # sync queue starts with g weights + chunks 1-7), then the
            # shared-expert feeds.
            es0 = ExitStack()
            rx0 = es0.enter_context(tc.tile_pool(name="rx0", bufs=1))
            xh0a = rx0.tile([P, KT_H // 2, TCH], BF16, name="xh0a")
            xh0b = rx0.tile([P, KT_H // 2, TCH], BF16, name="xh0b")
            xl0a = rx0.tile([P, KT_H // 2, TCH], BF16, name="xl0a")
            xl0b = rx0.tile([P, KT_H // 2, TCH], BF16, name="xl0b")
            nc.scalar.dma_start(out=xh0a, in_=_c3(xhi_d.ap()[0], TCH)[:, 0:KT_H // 2, :])
            nc.sync.dma_start(out=xh0b, in_=_c3(xhi_d.ap()[0], TCH)[:, KT_H // 2:, :])
            nc.gpsimd.dma_start(out=xl0a, in_=_c3(xlo_d.ap()[0], TCH)[:, 0:KT_H // 2, :])
            nc.scalar.dma_start(out=xl0b, in_=_c3(xlo_d.ap()[0], TCH)[:, KT_H // 2:, :])
            # shared expert feeds on scalar queue
            xs = sres.tile([P, KT_H, TS], BF16, name="xs")
            nc.scalar.dma_start(out=xs, in_=_c3(xbs_d.ap(), TS))
            # sw13 loads self-throttle via the swx slot rotation (later
            # tiles wait on earlier shared-A blocks); sw2 queues behind
            # them so its 4MB stays out of the router-chunk congestion
            # window but lands before shared-C needs it.
            sw1q_t, sw3q_t, sw2q_t = {}, {}, {}
            for q in range(4):
                sw1q_t[q] = swp.tile([P, KT_H, 256], BF16, name="sw1q",
                                     tag="swx", bufs=2)
                sw3q_t[q] = swp.tile([P, KT_H, 256], BF16, name="sw3q",
                                     tag="swx", bufs=2)
                nc.scalar.dma_start(out=sw1q_t[q], in_=_c3(sw13_d.ap()[0, q], 256))
                nc.scalar.dma_start(out=sw3q_t[q], in_=_c3(sw13_d.ap()[1, q], 256))
            for q in range(4):
                sw2q_t[q] = swp.tile([P, KT_I, 512], BF16, name="sw2q",
                                     tag="sw2", bufs=4)
                nc.scalar.dma_start(out=sw2q_t[q], in_=_c3(sw2_d.ap()[q], 512))
            ys = sres.tile([P, KT_I, TS], BF16, name="ys")

            def shared_a_block(mi, use_sigmoid):
                h, m = mi // 2, mi % 2
                sw1h, sw3h = sw1q_t[h], sw3q_t[h]
                msl = slice(m * P, (m + 1) * P)
                pg = aps.tile([P, ACH], F32, name="spg",
                              tag=f"pg{mi % 2}")[:, :TS]
                pu = aps.tile([P, ACH], F32, name="spu",
                              tag=f"pu{mi % 2}")[:, :TS]
                for kt in range(KT_H):
                    nc.tensor.matmul(pg, sw1h[:, kt, msl], xs[:, kt, :],
                                     start=(kt == 0), stop=(kt == KT_H - 1))
                for kt in range(KT_H):
                    nc.tensor.matmul(pu, sw3h[:, kt, msl], xs[:, kt, :],
                                     start=(kt == 0), stop=(kt == KT_H - 1))
                sg = so.tile([P, TS], BF16, name="ssg", tag="ssg")
                if use_sigmoid:
                    # silu(x) = x * sigmoid(x): avoids Sigmoid<->Silu
                    # ACT-table reloads between router chunks
                    nc.scalar.activation(sg, pg, AF.Sigmoid)
                    st = so.tile([P, TS], BF16, name="sst", tag="sst")
                    nc.vector.tensor_tensor(st, sg, pu, ALU.mult)
                    nc.vector.tensor_tensor(ys[:, mi, :], st, pg, ALU.mult)
                else:
                    nc.scalar.activation(sg, pg, AF.Silu)
                    nc.vector.tensor_tensor(ys[:, mi, :], sg, pu, ALU.mult)

            # =================== router (bf16 hi/lo) ===================
            with tc.tile_pool(name="rt", bufs=2) as rt, \
                 tc.tile_pool(name="rt1", bufs=1) as rt1, \
                 tc.tile_pool(name="rxn", bufs=2) as rxn, \
                 tc.tile_pool(name="rtp", bufs=2, space="PSUM") as rtp:
                # gcat[:, kt, 0:16] = ghi, [:, kt, 32:48] = glo (16:32
                # zero pad): one M=48 stationary pass computes ghi@xh and
                # glo@xh together; the pad keeps glo's psum rows at base
                # partition 32 (engine partition-offset constraint)
                gcat = rt1.tile([P, KT_H, 3 * E], BF16, name="gcat")
                nc.sync.dma_start(out=gcat, in_=_c3(g_d.ap(), 3 * E))
                biasb = rt1.tile([P, E], F32, name="biasb")
                nc.sync.dma_start(out=biasb, in_=biasb_d.ap())
                esel = rt1.tile([P, 2, E], F32, name="esel")
                nc.sync.dma_start(out=esel, in_=esel_d.ap())
                sT = rt1.tile([16, T], F32, name="sT")

                def epilogue_part(ts0, nts):
                    """Grouped top-k for tt in [ts0, ts0+nts) -> C2/M2."""
                    tsl = slice(ts0, ts0 + nts)
                    sc = sc_all[:, tsl, :]
                    selA = rt.tile([P, 8, E], F32, name="selA",
                                   tag="selA")[:, :nts, :]
                    nc.vector.tensor_tensor(
                        selA, sc,
                        biasb[:, None, :].broadcast_to([P, nts, E]), ALU.add)
                    a = selA[:, :, 0::4]
                    b = selA[:, :, 1::4]
                    c_ = selA[:, :, 2::4]
                    d = selA[:, :, 3::4]
                    t4 = rt.tile([P, 8, 6, G], F32, name="t4",
                                 tag="t4")[:, :nts, :, :]
                    m1, n1, m2, n2, gs, tmp = (t4[:, :, j, :] for j in range(6))
                    nc.vector.tensor_tensor(m1, a, b, ALU.max)
                    nc.vector.tensor_tensor(n1, a, b, ALU.min)
                    nc.vector.tensor_tensor(m2, c_, d, ALU.max)
                    nc.vector.tensor_tensor(n2, c_, d, ALU.min)
                    nc.vector.tensor_tensor(gs, m1, m2, ALU.add)
                    nc.vector.tensor_tensor(tmp, m1, n1, ALU.add)
                    nc.vector.tensor_tensor(gs, gs, tmp, ALU.max)
                    nc.vector.tensor_tensor(tmp, m2, n2, ALU.add)
                    nc.vector.tensor_tensor(gs, gs, tmp, ALU.max)
                    g2 = rt.tile([P, 8, 6], F32, name="g2",
                                 tag="g2")[:, :nts, :]
                    ga, gb = gs[:, :, 0::2], gs[:, :, 1::2]
                    gmx, gmn = g2[:, :, 0:2], g2[:, :, 2:4]
                    gthr = g2[:, :, 4:5]
                    gt2 = g2[:, :, 5:6]
                    nc.vector.tensor_tensor(gmx, ga, gb, ALU.max)
                    nc.vector.tensor_tensor(gmn, ga, gb, ALU.min)
                    nc.vector.tensor_tensor(gthr, gmx[:, :, 0:1], gmx[:, :, 1:2],
                                            ALU.min)
                    nc.vector.tensor_tensor(gt2, gmn[:, :, 0:1], gmn[:, :, 1:2],
                                            ALU.max)
                    nc.vector.tensor_tensor(gthr, gthr, gt2, ALU.max)
                    gmask = rt.tile([P, 8, G], F32, name="gmask",
                                    tag="gmask")[:, :nts, :]
                    nc.vector.tensor_tensor(
                        gmask, gs, gthr.broadcast_to([P, nts, G]), ALU.is_ge)
                    emask = rt.tile([P, 8, E], F32, name="emask",
                                    tag="emask")[:, :nts, :]
                    for j in range(4):
                        nc.vector.tensor_copy(emask[:, :, j::4], gmask)
                    masked = rt.tile([P, 8, E], F32, name="masked",
                                     tag="masked")[:, :nts, :]
                    nc.vector.tensor_scalar_add(emask, emask, -1.0)
                    nc.vector.scalar_tensor_tensor(masked, emask, 1e30, selA,
                                                   ALU.mult, ALU.add)
                    m8s = rt.tile([P, 8, 8], F32, name="m8s",
                                  tag="m8s")[:, :nts, :]
                    for tt in range(nts):
                        nc.vector.max(m8s[:, tt, :], masked[:, tt, :])
                    selm = rt.tile([P, 8, E], F32, name="selm",
                                   tag="selm")[:, :nts, :]
                    nc.vector.tensor_tensor(
                        selm, masked,
                        m8s[:, :, 3:4].broadcast_to([P, nts, E]), ALU.is_ge)
                    cw = rt.tile([P, 8, E], F32, name="cw",
                                 tag="cw")[:, :nts, :]
                    nc.vector.tensor_tensor(cw, sc, selm, ALU.mult)
                    den = rt.tile([P, 8, 2], F32, name="den",
                                  tag="den")[:, :nts, :]
                    nc.vector.reduce_sum(den[:, :, 0:1], cw, AX)
                    nc.vector.tensor_scalar_add(den[:, :, 0:1], den[:, :, 0:1],
                                                1e-20)
                    nc.vector.reciprocal(den[:, :, 1:2], den[:, :, 0:1])
                    nc.vector.tensor_scalar_mul(den[:, :, 1:2], den[:, :, 1:2],
                                                ROUTED_SCALE)
                    nc.vector.tensor_tensor(
                        cw, cw, den[:, :, 1:2].broadcast_to([P, nts, E]), ALU.mult)
                    esm = rt.tile([P, 8, E], F32, name="esm",
                                  tag="esm")[:, :nts, :]
                    for k in range(E_PER_CORE):
                        nc.vector.tensor_tensor(
                            esm, cw,
                            esel[:, k, :][:, None, :].broadcast_to([P, nts, E]),
                            ALU.mult)
                        nc.vector.reduce_sum(C2_sb[:, tsl, k:k + 1], esm, AX)
                    nc.vector.tensor_scalar(
                        M2_sb[:, tsl, :].rearrange("p a b -> p (a b)"),
                        C2_sb[:, tsl, :].rearrange("p a b -> p (a b)"),
                        0.0, None, ALU.is_gt)

                for n in range(NTOK):
                    if n == 0:
                        xh_parts = [(xh0a, 0), (xh0b, KT_H // 2)]
                        xl_parts = [(xl0a, 0), (xl0b, KT_H // 2)]
                    else:
                        xh = rxn.tile([P, KT_H, TCH], BF16, name="xh",
                                      tag="xh", bufs=2)
                        xl = rxn.tile([P, KT_H, TCH], BF16, name="xl",
                                      tag="xl", bufs=1)
                        nc.sync.dma_start(out=xh, in_=_c3(xhi_d.ap()[n], TCH))
                        nc.gpsimd.dma_start(out=xl, in_=_c3(xlo_d.ap()[n], TCH))
                        xh_parts = [(xh, 0)]
                        xl_parts = [(xl, 0)]
                    tksl = slice(n * TCH, (n + 1) * TCH)
                    ps = rtp.tile([48, TCH], F32, name="ps_r", tag="ps_r")
                    # pass 1: [ghi|glo] @ xh -> rows 0:32; pass 2:
                    # ghi @ xl accumulates into rows 0:16
                    for pi, (x_, koff) in enumerate(xh_parts):
                        nkt = x_.shape[1]
                        for kt in range(nkt):
                            nc.tensor.matmul(
                                ps, gcat[:, koff + kt, :], x_[:, kt, :],
                                start=(pi == 0 and kt == 0), stop=False)
                    nl = len(xl_parts)
                    for pi, (x_, koff) in enumerate(xl_parts):
                        nkt = x_.shape[1]
                        for kt in range(nkt):
                            nc.tensor.matmul(
                                ps[0:16, :], gcat[:, koff + kt, 0:E],
                                x_[:, kt, :],
                                start=False,
                                stop=(pi == nl - 1 and kt == nkt - 1))
                    s2 = rt.tile([16, 2, TCH], F32, name="s2", tag="s2")
                    nc.vector.tensor_copy(s2[:, 1, :], ps[32:48, :])
                    nc.vector.tensor_tensor(s2[:, 0, :], ps[0:16, :],
                                            s2[:, 1, :], ALU.add)
                    nc.scalar.activation(sT[:, tksl], s2[:, 0, :], AF.Sigmoid)
                    for tt in range(2 * n, 2 * n + 2):
                        pst = rtp.tile([P, 16], F32, name="pst", tag="pst")
                        nc.tensor.transpose(pst, sT[:, tt * P:(tt + 1) * P],
                                            ident[:16, :16])
                        nc.vector.tensor_copy(sc_all[:, tt, :], pst)
                    if n < N_SA_ROUTER:
                        shared_a_block(n, use_sigmoid=True)
                    if n == 3:
                        epilogue_part(0, 8)
                    elif n == 5:
                        epilogue_part(8, 4)
                epilogue_part(12, 4)
            es0.close()   # free chunk-0 tiles before FFN pools allocate

            # ============ compaction + shared + routed FFN ============
            # PSUM banks (8): aps 4 (pg0,pg1,pu0,pu1; shared-A + routed A),
            # zc 4 (pz0..pz3): shared-C on pz0/pz1, compaction accum on
            # pz2/pz3, routed C cycles all four.
            with tc.tile_pool(name="cmp", bufs=1) as cmp, \
                 tc.tile_pool(name="cmp1", bufs=2) as cmp1, \
                 tc.tile_pool(name="zc", bufs=1, space="PSUM") as zc, \
                 tc.tile_pool(name="aw", bufs=4) as aw, \
                 tc.tile_pool(name="w2p", bufs=2) as w2p, \
                 tc.tile_pool(name="ay", bufs=2) as ay, \
                 tc.tile_pool(name="ag", bufs=2) as ag, \
                 tc.tile_pool(name="zo", bufs=2) as zo:

                # sync (SP) HWDGE queue (behind router xhi stream):
                # routed weights, ordered by first need
                w1h_t, w3h_t, w2h_t = {}, {}, {}

                def _w13(k, h):
                    w1h = aw.tile([P, KT_H, 512], BF16, name="w1h", tag="wA")
                    w3h = aw.tile([P, KT_H, 512], BF16, name="w3h", tag="wA")
                    nc.sync.dma_start(out=w1h, in_=_c3(w13_d.ap()[k, h, 0], 512))
                    nc.sync.dma_start(out=w3h, in_=_c3(w13_d.ap()[k, h, 1], 512))
                    w1h_t[(k, h)] = w1h
                    w3h_t[(k, h)] = w3h

                def _w2(k, q):
                    w2q = w2p.tile([P, KT_I, 512], BF16, name="w2q", tag="w2")
                    nc.sync.dma_start(out=w2q, in_=_c3(w2_d.ap()[k, q], 512))
                    w2h_t[(k, q)] = w2q

                _w13(0, 0)
                _w13(0, 1)
                _w2(0, 0)
                _w2(0, 1)
                _w13(1, 0)
                _w13(1, 1)
                _w2(0, 2)
                _w2(0, 3)
                for q in range(4):
                    _w2(1, q)

                # ---- compaction: rank chains + batched mask build ----
                # phase a (both experts): rank via cumsum matmuls + scan
                # + digit split; then per expert: one-hot masks for ALL
                # 16 token tiles in a few large vector ops (fp16),
                # scatter matmuls, idx broadcast, gathers.  shared-A
                # block 6 leads so the PE has work while the vector
                # engine runs the final epilogue piece.
                shared_a_block(N_SA_ROUTER, use_sigmoid=False)
                digs, c16s = [], []
                for k in range(E_PER_CORE):
                    M = M2_sb[:, :, k]
                    cum_t = zc.tile([P, NC16], F32, name="cum_t",
                                    tag="pz2")[:, 0:TT]
                    cmt = zc.tile([P, NC16], F32, name="cmt", tag="pz3")
                    tot_ps = cmt[0:1, 0:TT]
                    nc.tensor.matmul(cum_t, tril, M, start=True, stop=True)
                    nc.tensor.matmul(tot_ps, ones128p, M, start=True, stop=True)
                    tot = cmp1.tile([1, 3, TT], F32, name="tot", tag="tot")
                    ex0, ex1 = tot[:, 1, :], tot[:, 2, :]
                    nc.vector.tensor_copy(tot[:, 0, :], tot_ps)
                    nc.vector.memset(ex0[:, 0:1], 0.0)
                    nc.vector.tensor_copy(ex0[:, 1:], tot[:, 0, 0:TT - 1])
                    nc.vector.tensor_tensor_scan(ex1, ex0, ex0, 0.0,
                                                 ALU.add, ALU.bypass)
                    carry_ps = cmt[:, TT:2 * TT]
                    nc.tensor.matmul(carry_ps, ones_row, ex1, start=True, stop=True)
                    # rank, with non-routed tokens pushed out of range
                    # (+2048: keeps rank%16, sends rank//16 beyond 47, so
                    # they scatter to nothing -- no separate mask mult)
                    rank = cmp1.tile([P, TT], F32, name="rank", tag="rank")
                    nc.vector.tensor_tensor(rank, cum_t, M, ALU.subtract)
                    nc.vector.tensor_tensor(rank, rank, carry_ps, ALU.add)
                    nc.vector.tensor_scalar_add(rank, rank, 2048.0)
                    nc.vector.scalar_tensor_tensor(rank, M, -2048.0, rank,
                                                   ALU.mult, ALU.add)
                    rank_i = cmp1.tile([P, TT], I32, name="rank_i", tag="rank_i")
                    nc.vector.tensor_copy(rank_i, rank)
                    digi = cmp1.tile([P, 2, TT], I32, name="digi", tag="digi")
                    nc.vector.tensor_scalar(digi[:, 0, :], rank_i, 15, None,
                                            ALU.bitwise_and)
                    nc.vector.tensor_scalar(digi[:, 1, :], rank_i, 4, None,
                                            ALU.logical_shift_right)
                    dig = cmp1.tile([P, 2, TT], F16, name="dig", tag="dig")
                    nc.vector.tensor_copy(dig, digi)
                    digs.append(dig)
                    c16 = cmp1.tile([P, TT], F16, name="c16", tag="c16")
                    nc.vector.tensor_copy(c16, C2_sb[:, :, k])
                    c16s.append(c16)

                def compact_mms(k):
                    """Mask build + accumulating scatter matmuls + idx."""
                    dig = digs[k]
                    s16m = cmp.tile([P, TT, 16], F16, name=f"s16_{k}",
                                    tag=f"s16_{k}")
                    m48t = cmp.tile([P, TT, NC16], F16, name=f"m48a_{k}",
                                    tag=f"m48a_{k}")
                    m48c = cmp.tile([P, TT, NC16], F16, name=f"m48b_{k}",
                                    tag=f"m48b_{k}")
                    lo_b = dig[:, 0, :, None].broadcast_to([P, TT, 16])
                    hi_b = dig[:, 1, :, None].broadcast_to([P, TT, NC16])
                    C_b = c16s[k][:, :, None].broadcast_to([P, TT, NC16])
                    nc.vector.tensor_tensor(s16m, i16r, lo_b, ALU.is_equal)
                    # eq48 = (iota48 == rank//16); m48c = eq48*C (separate
                    # tile), then m48t *= tokid in place
                    nc.vector.tensor_tensor(m48t, i48r, hi_b, ALU.is_equal)
                    nc.vector.tensor_tensor(m48c, m48t, C_b, ALU.mult)
                    nc.vector.tensor_tensor(m48t, m48t, tokr, ALU.mult)
                    ids_t = zc.tile([P, NC16], F32, name="ids_t",
                                    tag="pz2")[0:16, :]
                    w_t = zc.tile([P, NC16], F32, name="w_t",
                                  tag="pz3")[0:16, :]
                    for tt in range(TT):
                        nc.tensor.matmul(ids_t, s16m[:, tt, :],
                                         m48t[:, tt, :],
                                         start=(tt == 0), stop=(tt == TT - 1))
                        nc.tensor.matmul(w_t, s16m[:, tt, :],
                                         m48c[:, tt, :],
                                         start=(tt == 0), stop=(tt == TT - 1))
                    ids_f = cmp1.tile([16, NC16], F32, name="ids_f", tag="ids_f")
                    nc.vector.tensor_copy(ids_f, ids_t)
                    nc.vector.tensor_copy(W16[k], w_t)
                    bc_ps = zc.tile([P, NC16], F32, name="bc_ps", tag="pz3")
                    nc.tensor.matmul(bc_ps, BI, ids_f, start=True, stop=True)
                    nc.vector.tensor_copy(idx16[k], bc_ps)
                    # gathers for this expert start as soon as idx is ready
                    halves = []
                    for hh, hcap in enumerate(_halves(CAPS[k])):
                        base = hh * ACH
                        xgh = ag.tile([P, KT_H, hcap], BF16, name=f"xg{k}_{hh}",
                                      tag="xg", bufs=2)
                        csl = slice(base // 16, (base + hcap) // 16)
                        nc.gpsimd.dma_gather(
                            xgh, xbf_d.ap(), idx16[k][:, csl],
                            hcap, hcap, H, transpose=True)
                        halves.append(xgh)
                    return halves

                def shared_c_group(hc, s, gi):
                    sw2q = sw2q_t[hc]
                    hsl = slice(hc * 512, (hc + 1) * 512)
                    ssl = slice(s * P, (s + 1) * P)
                    pz = zc.tile([P, 512], F32, name="spz", tag=f"pz{gi % 2}")
                    for ki in range(KT_I):
                        nc.tensor.matmul(pz, ys[:, ki, ssl], sw2q[:, ki, :],
                                         start=(ki == 0), stop=(ki == KT_I - 1))
                    ot = so.tile([P, 512], BF16, name="ot", tag="ot")
                    nc.vector.tensor_copy(ot, pz)
                    nc.scalar.dma_start(out=out_d.ap()[ssl, hsl], in_=ot)

                # PE fill order across the compaction window:
                xg = [None, None]
                xg[0] = compact_mms(0)
                xg[1] = compact_mms(1)
                shared_a_block(N_SA_ROUTER + 1, use_sigmoid=False)
                gi = 0
                for hc in range(4):
                    for s in range(TS // P):
                        shared_c_group(hc, s, gi)
                        gi += 1
                # exports on the scalar HWDGE queue: keeps the Q7 queue
                # free for the critical-path gathers
                for k in range(E_PER_CORE):
                    nc.sync.dma_start(out=ids_d.ap()[k], in_=idx16[k][0:16, :])
                    # W16[q, 8s + r] -> W128[r*16 + q, s]
                    for r in range(8):
                        nc.sync.dma_start(out=W128[k][16 * r:16 * (r + 1), :],
                                          in_=W16[k][:, r::8])

                # ---------------- routed FFN per expert ----------------
                y = [sres.tile([P, KT_I, capc[k]], BF16, name=f"y{k}")
                     for k in range(E_PER_CORE)]
                for k in range(E_PER_CORE):
                    # pass A: y = silu(x@w1T) * (x@w3T) over COMPUTE cap
                    cchunks = [ACH, capc[k] - ACH]
                    for c, hcap in enumerate(cchunks):
                        for h in range(2):
                            w1h, w3h = w1h_t[(k, h)], w3h_t[(k, h)]
                            for m in range(4):
                                mi = h * 4 + m
                                msl = slice(m * P, (m + 1) * P)
                                pg = aps.tile([P, ACH], F32, name="pg",
                                              tag=f"pg{m % 2}")[:, :hcap]
                                pu = aps.tile([P, ACH], F32, name="pu",
                                              tag=f"pu{m % 2}")[:, :hcap]
                                for kt in range(KT_H):
                                    nc.tensor.matmul(
                                        pg, w1h[:, kt, msl],
                                        xg[k][c][:, kt, :hcap],
                                        start=(kt == 0), stop=(kt == KT_H - 1))
                                for kt in range(KT_H):
                                    nc.tensor.matmul(
                                        pu, w3h[:, kt, msl],
                                        xg[k][c][:, kt, :hcap],
                                        start=(kt == 0), stop=(kt == KT_H - 1))
                                sg = ay.tile([P, ACH], BF16, name="sg",
                                             tag="sg")[:, :hcap]
                                nc.scalar.activation(sg, pg, AF.Silu)
                                csl = slice(c * ACH, c * ACH + hcap)
                                nc.vector.tensor_tensor(y[k][:, mi, csl], sg,
                                                        pu, ALU.mult)
                    # pass C: z = W * (y @ w2T); groups cycle 4 psum banks
                    gi = 0
                    ns_full, rem = divmod(capc[k], P)
                    sdims = [P] * ns_full + ([rem] if rem else [])
                    for q in range(4):
                        w2q = w2h_t[(k, q)]
                        hsl = slice(q * 512, (q + 1) * 512)
                        for s, sdim in enumerate(sdims):
                            ssl = slice(s * P, s * P + sdim)
                            pz = zc.tile([P, 512], F32, name="pz",
                                         tag=f"pz{gi % 4}")[:sdim, :]
                            gi += 1
                            for ki in range(KT_I):
                                nc.tensor.matmul(pz, y[k][:, ki, ssl],
                                                 w2q[:, ki, :],
                                                 start=(ki == 0),
                                                 stop=(ki == KT_I - 1))
                            zt = zo.tile([P, 512], BF16, name="zc",
                                         tag="zc")[:sdim, :]
                            nc.vector.tensor_scalar_mul(zt, pz,
                                                        W128[k][:sdim, s:s + 1])
                            nc.sync.dma_start(out=z_d.ap()[k, ssl, hsl],
                                               in_=zt)

            es_.close()

    nc.compile()
    return nc


_NC_CACHE = {}


def _get_nc(capc):
    if capc not in _NC_CACHE:
        _NC_CACHE[capc] = build_nc(capc)
    return _NC_CACHE[capc]


def _route_counts(x, gate_w, expert_bias):
    """Host-side routing counts, used ONLY for load-balanced expert->core
    assignment and compute-capacity sizing (sharding decisions); the
    device recomputes routing."""
    logits = x @ gate_w.T
    scores = 1.0 / (1.0 + np.exp(-logits))
    sel = scores + expert_bias[None, :]
    grp = sel.reshape(T, G, E // G)
    t2 = np.sort(grp, -1)[:, :, -2:].sum(-1)
    gidx = np.argsort(t2, -1)[:, -2:]
    gmask = np.zeros((T, G), bool)
    gmask[np.arange(T)[:, None], gidx] = True
    emask = np.repeat(gmask, E // G, axis=1)
    masked = np.where(emask, sel, -np.inf)
    ids = np.argsort(masked, -1)[:, -K_TOP:]
    return np.bincount(ids.ravel(), minlength=E)


def _pc(aT, ncol):
    """[KT*128, C] -> [C//ncol, 128, KT*ncol] partition-contiguous."""
    kt = aT.shape[0] // P
    nch = aT.shape[1] // ncol
    a = aT.reshape(kt, P, nch, ncol)
    return np.ascontiguousarray(
        np.transpose(a, (2, 1, 0, 3)).reshape(nch, P, kt * ncol))


def kernel(hidden_states, gate_w, expert_bias, w1, w3, w2, sw1, sw3, sw2):
    x = np.ascontiguousarray(hidden_states, dtype=np.float32)
    bf = ml_dtypes.bfloat16
    xhi = x.astype(bf)
    xlo = (x - xhi.astype(np.float32)).astype(bf)
    gw = np.ascontiguousarray(gate_w.astype(np.float32))
    ghi = gw.astype(bf)
    glo = (gw - ghi.astype(np.float32)).astype(bf)
    xbf = np.ascontiguousarray(xhi)
    xhiT = np.ascontiguousarray(xhi.T)
    xloT = np.ascontiguousarray(xlo.T)
    bias = expert_bias.astype(np.float32)
    biasb = np.ascontiguousarray(np.broadcast_to(bias[None, :], (P, E)))

    # partition-contiguous streaming layouts
    xhi_r = _pc(xhiT, TCH)                      # [NTOK, P, KT_H*TCH]
    xlo_r = _pc(xloT, TCH)
    gpair = np.concatenate([np.ascontiguousarray(ghi.T),
                            np.zeros((H, E), ghi.dtype),
                            np.ascontiguousarray(glo.T)], axis=1)
    g_r = _pc(np.ascontiguousarray(gpair), 3 * E)[0]
    sw13_r = np.stack([_pc(np.ascontiguousarray(sw1.T.astype(bf)), 256),
                       _pc(np.ascontiguousarray(sw3.T.astype(bf)), 256)])
    sw2_r = _pc(np.ascontiguousarray(sw2.T.astype(bf)), 512)

    # load-balanced assignment: pair i-th largest with i-th smallest
    counts = _route_counts(x.astype(np.float64), gw.astype(np.float64),
                           bias.astype(np.float64))
    order = np.argsort(-counts)
    assign = [(int(order[i]), int(order[E - 1 - i])) for i in range(N_CORES)]
    # compute capacity: actual max per slot + margin, rounded to 8
    cnt0 = max(counts[a] for a, _ in assign)
    cnt1 = max(counts[b] for _, b in assign)
    capc = (min(CAPS[0], max(ACH + 8, -(-(int(cnt0) + 8) // 8) * 8)),
            min(CAPS[1], max(ACH + 8, -(-(int(cnt1) + 8) // 8) * 8)))

    w1tb = np.transpose(w1, (0, 2, 1)).astype(bf)
    w3tb = np.transpose(w3, (0, 2, 1)).astype(bf)
    w2tb = np.transpose(w2, (0, 2, 1)).astype(bf)

    in_maps = []
    for c in range(N_CORES):
        e_hi, e_lo = assign[c]
        esel = np.zeros((P, 2, E), np.float32)
        esel[:, 0, e_hi] = 1.0
        esel[:, 1, e_lo] = 1.0
        pick = [e_hi, e_lo]
        w13_r = np.stack([
            np.stack([
                np.stack([_pc(np.ascontiguousarray(w1tb[e]), 512),
                          _pc(np.ascontiguousarray(w3tb[e]), 512)], axis=1)[h]
                for h in range(2)])
            for e in pick])                     # [2, 2, 2, P, KT_H*512]
        w2_r = np.stack([_pc(np.ascontiguousarray(w2tb[e]), 512)
                         for e in pick])        # [2, 4, P, KT_I*512]
        xbs_r = _pc(np.ascontiguousarray(xhiT[:, TS * c:TS * (c + 1)]), TS)[0]
        in_maps.append({
            "xhi": xhi_r,
            "xlo": xlo_r,
            "xbf": xbf,
            "g": g_r,
            "biasb": biasb,
            "esel": esel,
            "w13": np.ascontiguousarray(w13_r),
            "w2": np.ascontiguousarray(w2_r),
            "sw13": sw13_r,
            "sw2": sw2_r,
            "xbs": xbs_r,
        })

    nc = _get_nc(capc)
    res = run_bass_kernel_spmd(nc, in_maps, list(range(N_CORES)))

    out = np.zeros((T, H), np.float32)
    for c in range(N_CORES):
        r = res.results[c]
        z = np.asarray(r["z"], dtype=np.float32)          # [2, CAPS[0], H]
        ids = np.asarray(r["ids"], dtype=np.int64)        # [2, 16, NC16]
        for k in range(E_PER_CORE):
            slot_ids = ids[k].T.reshape(-1)               # slot i at [i%16, i//16]
            nz = np.nonzero(slot_ids)[0]
            cnt = (nz[-1] + 1) if len(nz) else 0
            cnt = min(cnt, capc[k])
            if cnt:
                out[slot_ids[:cnt]] += z[k, :cnt]
        out[TS * c:TS * (c + 1)] += np.asarray(r["out"], dtype=np.float32)
    kernel.last_result = res
    return out


# revision 40
# speedup vs baseline: 1.1014x; 1.1014x over previous
"""MoE (BailingMoeV2.5) Trainium2 kernel — 8-core expert-parallel, SPARSE.

T=2048 tokens, H=2048 hidden, E=16 experts (4 groups, top-2 groups,
top-4 experts), I=1024 expert intermediate, shared expert IS=1024,
routed scale 2.5.

Each core owns 2 experts (host pairs high-count with low-count experts;
gather slot capacities 768/640, compute capacities trimmed to the
actual max routed counts + margin, rounded to 8):
  1. Router: logits via lossless-ish bf16 hi/lo split (3 bf16 passes),
     sigmoid scores, batched grouped top-k epilogue (3 pieces: 8/4/4
     token-tiles, overlapped with the score stream) -> per-token
     combine weights C2[token, 2] for this core's experts (x2.5,
     renormalized).
  2. Device-side stream compaction per expert (cumsum-matmul rank +
     batched one-hot mask build via big tensor_tensor ops + fused
     onehot matmuls) -> token-id list (int16, dma_gather layout) +
     per-slot weights.  Padding slots gather token 0 with W=0.
  3. dma_gather (transpose mode) pulls selected tokens from the bf16
     token-major x into feature-major [128, 16, cap].
  4. bf16 SwiGLU FFN per expert over the COMPUTE capacity only;
     output scaled by W -> z + ids exported.
  5. Shared expert (bf16) on the core's 256-token slice; shared-A
     blocks 0-4 interleave with router chunks (silu computed as
     x*sigmoid(x) to avoid ACT-table thrash), blocks 5-7 + shared
     pass C fill the compaction window.
Host unshard: out[ids] += z per (core, slot); out[slice_c] += shared_c.

DMA queues: sync(SP) = xhi router chunks then routed weights;
scalar(Act) = chunk-0 split + shared-expert feeds then outputs;
gpsimd = xlo router chunks, idx bookkeeping + gathers.
All large streams use host-side partition-contiguous layouts
(one >=4KB contiguous run per partition per DMA).
"""
import sys
from contextlib import ExitStack

sys.path.insert(0, "/opt/trn_rl_repo")

import numpy as np
import ml_dtypes

import concourse.bass as bass
import concourse.mybir as mybir
import concourse.tile as tile
from concourse import bacc
from concourse.bass_utils import run_bass_kernel_spmd
from concourse.masks import make_identity, make_upper_triangular

P = 128
T, H, E, K_TOP, I = 2048, 2048, 16, 4, 1024
G = 4
IS = 1024
N_CORES = 8
E_PER_CORE = E // N_CORES  # 2
TS = T // N_CORES          # 256
ROUTED_SCALE = 2.5

KT_H = H // P              # 16
KT_I = I // P              # 8
NTOK = 8                   # router token chunks of 256
TCH = T // NTOK            # 256
TT = T // P                # 16
CAPS = (768, 640)          # gather slot capacity (multiple of 128)
NC16 = 48                  # idx cols allocated (CAPS[0]/16)
ACH = 384                  # pass-A slot chunk (psum bank fits 384 fp32)
N_SA_ROUTER = 6            # shared-A blocks interleaved with router

F32 = mybir.dt.float32
F16 = mybir.dt.float16
BF16 = mybir.dt.bfloat16
I16 = mybir.dt.int16
I32 = mybir.dt.int32
AX = mybir.AxisListType.X
ALU = mybir.AluOpType
AF = mybir.ActivationFunctionType


def _halves(cap):
    return (ACH, cap - ACH)


def build_nc(capc):
    """capc: per-slot COMPUTE capacity (<= CAPS, multiple of 8)."""
    nc = bacc.Bacc(None, target_bir_lowering=False, debug=False)

    # all streaming inputs are partition-contiguous: [.., P, inner]
    xhi_d = nc.declare_dram_parameter("xhi", [NTOK, P, KT_H * TCH], BF16, isOutput=False)
    xlo_d = nc.declare_dram_parameter("xlo", [NTOK, P, KT_H * TCH], BF16, isOutput=False)
    xbf_d = nc.declare_dram_parameter("xbf", [T, H], BF16, isOutput=False)
    g_d = nc.declare_dram_parameter("g", [P, KT_H * 3 * E], BF16, isOutput=False)
    biasb_d = nc.declare_dram_parameter("biasb", [P, E], F32, isOutput=False)
    esel_d = nc.declare_dram_parameter("esel", [P, 2, E], F32, isOutput=False)
    w13_d = nc.declare_dram_parameter("w13", [E_PER_CORE, 2, 2, P, KT_H * 512], BF16, isOutput=False)
    w2_d = nc.declare_dram_parameter("w2", [E_PER_CORE, 4, P, KT_I * 512], BF16, isOutput=False)
    sw13_d = nc.declare_dram_parameter("sw13", [2, 4, P, KT_H * 256], BF16, isOutput=False)
    sw2_d = nc.declare_dram_parameter("sw2", [4, P, KT_I * 512], BF16, isOutput=False)
    xbs_d = nc.declare_dram_parameter("xbs", [P, KT_H * TS], BF16, isOutput=False)

    z_d = nc.declare_dram_parameter("z", [E_PER_CORE, CAPS[0], H], BF16, isOutput=True)
    ids_d = nc.declare_dram_parameter("ids", [E_PER_CORE, 16, NC16], I16, isOutput=True)
    out_d = nc.declare_dram_parameter("out", [TS, H], BF16, isOutput=True)

    def _c3(ap, n):
        return ap.rearrange("p (kt n) -> p kt n", n=n)

    with tile.TileContext(nc) as tc:
        with tc.tile_pool(name="res", bufs=1) as res:
            # ---------------- persistent small tiles ----------------
            sc_all = res.tile([P, TT, E], F32, name="sc_all")
            C2_sb = res.tile([P, TT, E_PER_CORE], F32, name="C2_sb")
            M2_sb = res.tile([P, TT, E_PER_CORE], F32, name="M2_sb")
            ident = res.tile([P, P], F32, name="ident")
            make_identity(nc, ident)
            tril = res.tile([P, P], F32, name="tril")
            make_upper_triangular(nc, tril, val=1.0, diag=True)
            ones128p = res.tile([P, 1], F32, name="ones128p")
            nc.vector.memset(ones128p, 1.0)
            ones_row = res.tile([1, P], F32, name="ones_row")
            nc.vector.memset(ones_row, 1.0)
            iotas = res.tile([P, 80], F32, name="iotas")
            iota16 = iotas[:, 0:16]
            iota48 = iotas[:, 16:64]
            tokid = iotas[:, 64:80]
            ii = res.tile([P, NC16], I32, name="ii")
            nc.gpsimd.iota(ii[:, 0:16], pattern=[[1, 16]], base=0, channel_multiplier=0)
            nc.vector.tensor_copy(iota16, ii[:, 0:16])
            nc.gpsimd.iota(ii[:, 0:NC16], pattern=[[1, NC16]], base=0, channel_multiplier=0)
            nc.vector.tensor_copy(iota48, ii[:, 0:NC16])
            nc.gpsimd.iota(ii[:, 0:TT], pattern=[[P, TT]], base=0, channel_multiplier=1)
            nc.vector.tensor_copy(tokid, ii[:, 0:TT])

            idx16 = [res.tile([P, NC16], I16, name=f"idx16_{k}")
                     for k in range(E_PER_CORE)]
            W128 = [res.tile([P, 6], F32, name=f"W128_{k}")
                    for k in range(E_PER_CORE)]
            W16 = [res.tile([16, NC16], F32, name=f"W16_{k}")
                   for k in range(E_PER_CORE)]
            # iota/token-id repeats for the batched compaction masks,
            # built up-front while the engines are otherwise idle
            i16r = res.tile([P, TT, 16], F16, name="i16r")
            nc.vector.tensor_copy(
                i16r, iota16[:, None, :].broadcast_to([P, TT, 16]))
            i48r = res.tile([P, TT, NC16], F16, name="i48r")
            nc.vector.tensor_copy(
                i48r, iota48[:, None, :].broadcast_to([P, TT, NC16]))
            tokr = res.tile([P, TT, NC16], F16, name="tokr")
            nc.vector.tensor_copy(
                tokr, tokid[:, :, None].broadcast_to([P, TT, NC16]))
            # block-identity BI[q, p] = (p % 16 == q), for idx broadcast
            BI = res.tile([16, P], F32, name="BI")
            bii = res.tile([16, P], I32, name="bii")
            nc.gpsimd.iota(bii, pattern=[[1, P]], base=0, channel_multiplier=0)
            nc.vector.tensor_scalar(bii, bii, 15, None, ALU.bitwise_and)
            bif = res.tile([16, P], F32, name="bif")
            nc.vector.tensor_copy(bif, bii)
            qcolf = res.tile([16, 1], F32, name="qcolf")
            qcol = res.tile([16, 1], I32, name="qcol")
            nc.gpsimd.iota(qcol, pattern=[[1, 1]], base=0, channel_multiplier=1)
            nc.vector.tensor_copy(qcolf, qcol)
            nc.vector.tensor_scalar(BI, bif, qcolf, None, ALU.is_equal)

            # shared-expert pools at outer scope
            es_ = ExitStack()
            swp = es_.enter_context(tc.tile_pool(name="sw", bufs=3))
            sres = es_.enter_context(tc.tile_pool(name="sres", bufs=1))
            so = es_.enter_context(tc.tile_pool(name="so", bufs=2))
            aps = es_.enter_context(tc.tile_pool(name="aps", bufs=1, space="PSUM"))
            # scalar (Act) HWDGE queue: chunk-0 xhi halves first (the
            # sync queue starts with g weights + chunks 1-7), then the
            # shared-expert feeds.
            es0 = ExitStack()
            rx0 = es0.enter_context(tc.tile_pool(name="rx0", bufs=1))
            xh0a = rx0.tile([P, KT_H // 2, TCH], BF16, name="xh0a")
            xh0b = rx0.tile([P, KT_H // 2, TCH], BF16, name="xh0b")
            xl0a = rx0.tile([P, KT_H // 2, TCH], BF16, name="xl0a")
            xl0b = rx0.tile([P, KT_H // 2, TCH], BF16, name="xl0b")
            nc.scalar.dma_start(out=xh0a, in_=_c3(xhi_d.ap()[0], TCH)[:, 0:KT_H // 2, :])
            nc.sync.dma_start(out=xh0b, in_=_c3(xhi_d.ap()[0], TCH)[:, KT_H // 2:, :])
            nc.gpsimd.dma_start(out=xl0a, in_=_c3(xlo_d.ap()[0], TCH)[:, 0:KT_H // 2, :])
            nc.scalar.dma_start(out=xl0b, in_=_c3(xlo_d.ap()[0], TCH)[:, KT_H // 2:, :])
            # shared expert feeds on scalar queue
            xs = sres.tile([P, KT_H, TS], BF16, name="xs")
            nc.scalar.dma_start(out=xs, in_=_c3(xbs_d.ap(), TS))
            # sw13 loads self-throttle via the swx slot rotation (later
            # tiles wait on earlier shared-A blocks); sw2 queues behind
            # them so its 4MB stays out of the router-chunk congestion
            # window but lands before shared-C needs it.
            sw1q_t, sw3q_t, sw2q_t = {}, {}, {}
            for q in range(4):
                sw1q_t[q] = swp.tile([P, KT_H, 256], BF16, name="sw1q",
                                     tag="swx", bufs=2)
                sw3q_t[q] = swp.tile([P, KT_H, 256], BF16, name="sw3q",
                                     tag="swx", bufs=2)
                nc.scalar.dma_start(out=sw1q_t[q], in_=_c3(sw13_d.ap()[0, q], 256))
                nc.scalar.dma_start(out=sw3q_t[q], in_=_c3(sw13_d.ap()[1, q], 256))
            for q in range(4):
                sw2q_t[q] = swp.tile([P, KT_I, 512], BF16, name="sw2q",
                                     tag="sw2", bufs=4)
                nc.scalar.dma_start(out=sw2q_t[q], in_=_c3(sw2_d.ap()[q], 512))
            ys = sres.tile([P, KT_I, TS], BF16, name="ys")

            def shared_a_block(mi, use_sigmoid):
                h, m = mi // 2, mi % 2
                sw1h, sw3h = sw1q_t[h], sw3q_t[h]
                msl = slice(m * P, (m + 1) * P)
                pg = aps.tile([P, ACH], F32, name="spg",
                              tag=f"pg{mi % 2}")[:, :TS]
                pu = aps.tile([P, ACH], F32, name="spu",
                              tag=f"pu{mi % 2}")[:, :TS]
                for kt in range(KT_H):
                    nc.tensor.matmul(pg, sw1h[:, kt, msl], xs[:, kt, :],
                                     start=(kt == 0), stop=(kt == KT_H - 1))
                for kt in range(KT_H):
                    nc.tensor.matmul(pu, sw3h[:, kt, msl], xs[:, kt, :],
                                     start=(kt == 0), stop=(kt == KT_H - 1))
                sg = so.tile([P, TS], BF16, name="ssg", tag="ssg")
                if use_sigmoid:
                    # silu(x) = x * sigmoid(x): avoids Sigmoid<->Silu
                    # ACT-table reloads between router chunks
                    nc.scalar.activation(sg, pg, AF.Sigmoid)
                    st = so.tile([P, TS], BF16, name="sst", tag="sst")
                    nc.vector.tensor_tensor(st, sg, pu, ALU.mult)
                    nc.vector.tensor_tensor(ys[:, mi, :], st, pg, ALU.mult)
                else:
                    nc.scalar.activation(sg, pg, AF.Silu)
                    nc.vector.tensor_tensor(ys[:, mi, :], sg, pu, ALU.mult)

            # =================== router (bf16 hi/lo) ===================
            with tc.tile_pool(name="rt", bufs=2) as rt, \
                 tc.tile_pool(name="rt1", bufs=1) as rt1, \
                 tc.tile_pool(name="rxn", bufs=2) as rxn, \
                 tc.tile_pool(name="rtp", bufs=2, space="PSUM") as rtp:
                # gcat[:, kt, 0:16] = ghi, [:, kt, 32:48] = glo (16:32
                # zero pad): one M=48 stationary pass computes ghi@xh and
                # glo@xh together; the pad keeps glo's psum rows at base
                # partition 32 (engine partition-offset constraint)
                gcat = rt1.tile([P, KT_H, 3 * E], BF16, name="gcat")
                nc.sync.dma_start(out=gcat, in_=_c3(g_d.ap(), 3 * E))
                biasb = rt1.tile([P, E], F32, name="biasb")
                nc.sync.dma_start(out=biasb, in_=biasb_d.ap())
                esel = rt1.tile([P, 2, E], F32, name="esel")
                nc.sync.dma_start(out=esel, in_=esel_d.ap())
                sT = rt1.tile([16, T], F32, name="sT")

                def epilogue_part(ts0, nts):
                    """Grouped top-k for tt in [ts0, ts0+nts) -> C2/M2."""
                    tsl = slice(ts0, ts0 + nts)
                    sc = sc_all[:, tsl, :]
                    selA = rt.tile([P, 8, E], F32, name="selA",
                                   tag="selA")[:, :nts, :]
                    nc.vector.tensor_tensor(
                        selA, sc,
                        biasb[:, None, :].broadcast_to([P, nts, E]), ALU.add)
                    a = selA[:, :, 0::4]
                    b = selA[:, :, 1::4]
                    c_ = selA[:, :, 2::4]
                    d = selA[:, :, 3::4]
                    t4 = rt.tile([P, 8, 6, G], F32, name="t4",
                                 tag="t4")[:, :nts, :, :]
                    m1, n1, m2, n2, gs, tmp = (t4[:, :, j, :] for j in range(6))
                    nc.vector.tensor_tensor(m1, a, b, ALU.max)
                    nc.vector.tensor_tensor(n1, a, b, ALU.min)
                    nc.vector.tensor_tensor(m2, c_, d, ALU.max)
                    nc.vector.tensor_tensor(n2, c_, d, ALU.min)
                    nc.vector.tensor_tensor(gs, m1, m2, ALU.add)
                    nc.vector.tensor_tensor(tmp, m1, n1, ALU.add)
                    nc.vector.tensor_tensor(gs, gs, tmp, ALU.max)
                    nc.vector.tensor_tensor(tmp, m2, n2, ALU.add)
                    nc.vector.tensor_tensor(gs, gs, tmp, ALU.max)
                    g2 = rt.tile([P, 8, 6], F32, name="g2",
                                 tag="g2")[:, :nts, :]
                    ga, gb = gs[:, :, 0::2], gs[:, :, 1::2]
                    gmx, gmn = g2[:, :, 0:2], g2[:, :, 2:4]
                    gthr = g2[:, :, 4:5]
                    gt2 = g2[:, :, 5:6]
                    nc.vector.tensor_tensor(gmx, ga, gb, ALU.max)
                    nc.vector.tensor_tensor(gmn, ga, gb, ALU.min)
                    nc.vector.tensor_tensor(gthr, gmx[:, :, 0:1], gmx[:, :, 1:2],
                                            ALU.min)
                    nc.vector.tensor_tensor(gt2, gmn[:, :, 0:1], gmn[:, :, 1:2],
                                            ALU.max)
                    nc.vector.tensor_tensor(gthr, gthr, gt2, ALU.max)
                    gmask = rt.tile([P, 8, G], F32, name="gmask",
                                    tag="gmask")[:, :nts, :]
                    nc.vector.tensor_tensor(
                        gmask, gs, gthr.broadcast_to([P, nts, G]), ALU.is_ge)
                    emask = rt.tile([P, 8, E], F32, name="emask",
                                    tag="emask")[:, :nts, :]
                    for j in range(4):
                        nc.vector.tensor_copy(emask[:, :, j::4], gmask)
                    masked = rt.tile([P, 8, E], F32, name="masked",
                                     tag="masked")[:, :nts, :]
                    nc.vector.tensor_scalar_add(emask, emask, -1.0)
                    nc.vector.scalar_tensor_tensor(masked, emask, 1e30, selA,
                                                   ALU.mult, ALU.add)
                    m8s = rt.tile([P, 8, 8], F32, name="m8s",
                                  tag="m8s")[:, :nts, :]
                    for tt in range(nts):
                        nc.vector.max(m8s[:, tt, :], masked[:, tt, :])
                    selm = rt.tile([P, 8, E], F32, name="selm",
                                   tag="selm")[:, :nts, :]
                    nc.vector.tensor_tensor(
                        selm, masked,
                        m8s[:, :, 3:4].broadcast_to([P, nts, E]), ALU.is_ge)
                    cw = rt.tile([P, 8, E], F32, name="cw",
                                 tag="cw")[:, :nts, :]
                    nc.vector.tensor_tensor(cw, sc, selm, ALU.mult)
                    den = rt.tile([P, 8, 2], F32, name="den",
                                  tag="den")[:, :nts, :]
                    nc.vector.reduce_sum(den[:, :, 0:1], cw, AX)
                    nc.vector.tensor_scalar_add(den[:, :, 0:1], den[:, :, 0:1],
                                                1e-20)
                    nc.vector.reciprocal(den[:, :, 1:2], den[:, :, 0:1])
                    nc.vector.tensor_scalar_mul(den[:, :, 1:2], den[:, :, 1:2],
                                                ROUTED_SCALE)
                    nc.vector.tensor_tensor(
                        cw, cw, den[:, :, 1:2].broadcast_to([P, nts, E]), ALU.mult)
                    esm = rt.tile([P, 8, E], F32, name="esm",
                                  tag="esm")[:, :nts, :]
                    for k in range(E_PER_CORE):
                        nc.vector.tensor_tensor(
                            esm, cw,
                            esel[:, k, :][:, None, :].broadcast_to([P, nts, E]),
                            ALU.mult)
                        nc.vector.reduce_sum(C2_sb[:, tsl, k:k + 1], esm, AX)
                    nc.vector.tensor_scalar(
                        M2_sb[:, tsl, :].rearrange("p a b -> p (a b)"),
                        C2_sb[:, tsl, :].rearrange("p a b -> p (a b)"),
                        0.0, None, ALU.is_gt)

                for n in range(NTOK):
                    if n == 0:
                        xh_parts = [(xh0a, 0), (xh0b, KT_H // 2)]
                        xl_parts = [(xl0a, 0), (xl0b, KT_H // 2)]
                    else:
                        xh = rxn.tile([P, KT_H, TCH], BF16, name="xh",
                                      tag="xh", bufs=3)
                        xl = rxn.tile([P, KT_H, TCH], BF16, name="xl",
                                      tag="xl", bufs=1)
                        nc.sync.dma_start(out=xh, in_=_c3(xhi_d.ap()[n], TCH))
                        nc.gpsimd.dma_start(out=xl, in_=_c3(xlo_d.ap()[n], TCH))
                        xh_parts = [(xh, 0)]
                        xl_parts = [(xl, 0)]
                    tksl = slice(n * TCH, (n + 1) * TCH)
                    ps = rtp.tile([48, TCH], F32, name="ps_r", tag="ps_r")
                    # pass 1: [ghi|glo] @ xh -> rows 0:32; pass 2:
                    # ghi @ xl accumulates into rows 0:16
                    for pi, (x_, koff) in enumerate(xh_parts):
                        nkt = x_.shape[1]
                        for kt in range(nkt):
                            nc.tensor.matmul(
                                ps, gcat[:, koff + kt, :], x_[:, kt, :],
                                start=(pi == 0 and kt == 0), stop=False)
                    nl = len(xl_parts)
                    for pi, (x_, koff) in enumerate(xl_parts):
                        nkt = x_.shape[1]
                        for kt in range(nkt):
                            nc.tensor.matmul(
                                ps[0:16, :], gcat[:, koff + kt, 0:E],
                                x_[:, kt, :],
                                start=False,
                                stop=(pi == nl - 1 and kt == nkt - 1))
                    s2 = rt.tile([16, 2, TCH], F32, name="s2", tag="s2")
                    nc.vector.tensor_copy(s2[:, 1, :], ps[32:48, :])
                    nc.vector.tensor_tensor(s2[:, 0, :], ps[0:16, :],
                                            s2[:, 1, :], ALU.add)
                    nc.scalar.activation(sT[:, tksl], s2[:, 0, :], AF.Sigmoid)
                    for tt in range(2 * n, 2 * n + 2):
                        pst = rtp.tile([P, 16], F32, name="pst", tag="pst")
                        nc.tensor.transpose(pst, sT[:, tt * P:(tt + 1) * P],
                                            ident[:16, :16])
                        nc.vector.tensor_copy(sc_all[:, tt, :], pst)
                    if n < N_SA_ROUTER:
                        shared_a_block(n, use_sigmoid=True)
                    if n == 3:
                        epilogue_part(0, 8)
                    elif n == 5:
                        epilogue_part(8, 4)
                epilogue_part(12, 4)
            es0.close()   # free chunk-0 tiles before FFN pools allocate

            # ============ compaction + shared + routed FFN ============
            # PSUM banks (8): aps 4 (pg0,pg1,pu0,pu1; shared-A + routed A),
            # zc 4 (pz0..pz3): shared-C on pz0/pz1, compaction accum on
            # pz2/pz3, routed C cycles all four.
            with tc.tile_pool(name="cmp", bufs=1) as cmp, \
                 tc.tile_pool(name="cmp1", bufs=2) as cmp1, \
                 tc.tile_pool(name="zc", bufs=1, space="PSUM") as zc, \
                 tc.tile_pool(name="aw", bufs=4) as aw, \
                 tc.tile_pool(name="w2p", bufs=2) as w2p, \
                 tc.tile_pool(name="ay", bufs=2) as ay, \
                 tc.tile_pool(name="ag", bufs=2) as ag, \
                 tc.tile_pool(name="zo", bufs=2) as zo:

                # sync (SP) HWDGE queue (behind router xhi stream):
                # routed weights, ordered by first need
                w1h_t, w3h_t, w2h_t = {}, {}, {}

                def _w13(k, h):
                    w1h = aw.tile([P, KT_H, 512], BF16, name="w1h", tag="wA")
                    w3h = aw.tile([P, KT_H, 512], BF16, name="w3h", tag="wA")
                    nc.sync.dma_start(out=w1h, in_=_c3(w13_d.ap()[k, h, 0], 512))
                    nc.sync.dma_start(out=w3h, in_=_c3(w13_d.ap()[k, h, 1], 512))
                    w1h_t[(k, h)] = w1h
                    w3h_t[(k, h)] = w3h

                def _w2(k, q):
                    w2q = w2p.tile([P, KT_I, 512], BF16, name="w2q", tag="w2")
                    nc.sync.dma_start(out=w2q, in_=_c3(w2_d.ap()[k, q], 512))
                    w2h_t[(k, q)] = w2q

                _w13(0, 0)
                _w13(0, 1)
                _w2(0, 0)
                _w2(0, 1)
                _w13(1, 0)
                _w13(1, 1)
                _w2(0, 2)
                _w2(0, 3)
                for q in range(4):
                    _w2(1, q)

                # ---- compaction: rank chains + batched mask build ----
                # phase a (both experts): rank via cumsum matmuls + scan
                # + digit split; then per expert: one-hot masks for ALL
                # 16 token tiles in a few large vector ops (fp16),
                # scatter matmuls, idx broadcast, gathers.  shared-A
                # block 6 leads so the PE has work while the vector
                # engine runs the final epilogue piece.
                shared_a_block(N_SA_ROUTER, use_sigmoid=False)
                digs, c16s = [], []
                for k in range(E_PER_CORE):
                    M = M2_sb[:, :, k]
                    cum_t = zc.tile([P, NC16], F32, name="cum_t",
                                    tag="pz2")[:, 0:TT]
                    cmt = zc.tile([P, NC16], F32, name="cmt", tag="pz3")
                    tot_ps = cmt[0:1, 0:TT]
                    nc.tensor.matmul(cum_t, tril, M, start=True, stop=True)
                    nc.tensor.matmul(tot_ps, ones128p, M, start=True, stop=True)
                    tot = cmp1.tile([1, 3, TT], F32, name="tot", tag="tot")
                    ex0, ex1 = tot[:, 1, :], tot[:, 2, :]
                    nc.vector.tensor_copy(tot[:, 0, :], tot_ps)
                    nc.vector.memset(ex0[:, 0:1], 0.0)
                    nc.vector.tensor_copy(ex0[:, 1:], tot[:, 0, 0:TT - 1])
                    nc.vector.tensor_tensor_scan(ex1, ex0, ex0, 0.0,
                                                 ALU.add, ALU.bypass)
                    carry_ps = cmt[:, TT:2 * TT]
                    nc.tensor.matmul(carry_ps, ones_row, ex1, start=True, stop=True)
                    # rank, with non-routed tokens pushed out of range
                    # (+2048: keeps rank%16, sends rank//16 beyond 47, so
                    # they scatter to nothing -- no separate mask mult)
                    rank = cmp1.tile([P, TT], F32, name="rank", tag="rank")
                    nc.vector.tensor_tensor(rank, cum_t, M, ALU.subtract)
                    nc.vector.tensor_tensor(rank, rank, carry_ps, ALU.add)
                    nc.vector.tensor_scalar_add(rank, rank, 2048.0)
                    nc.vector.scalar_tensor_tensor(rank, M, -2048.0, rank,
                                                   ALU.mult, ALU.add)
                    rank_i = cmp1.tile([P, TT], I32, name="rank_i", tag="rank_i")
                    nc.vector.tensor_copy(rank_i, rank)
                    digi = cmp1.tile([P, 2, TT], I32, name="digi", tag="digi")
                    nc.vector.tensor_scalar(digi[:, 0, :], rank_i, 15, None,
                                            ALU.bitwise_and)
                    nc.vector.tensor_scalar(digi[:, 1, :], rank_i, 4, None,
                                            ALU.logical_shift_right)
                    dig = cmp1.tile([P, 2, TT], F16, name="dig", tag="dig")
                    nc.vector.tensor_copy(dig, digi)
                    digs.append(dig)
                    c16 = cmp1.tile([P, TT], F16, name="c16", tag="c16")
                    nc.vector.tensor_copy(c16, C2_sb[:, :, k])
                    c16s.append(c16)

                def compact_mms(k):
                    """Mask build + accumulating scatter matmuls + idx."""
                    dig = digs[k]
                    s16m = cmp.tile([P, TT, 16], F16, name=f"s16_{k}",
                                    tag="s16")
                    m48t = cmp.tile([P, TT, NC16], F16, name=f"m48a_{k}",
                                    tag="m48a")
                    m48c = cmp.tile([P, TT, NC16], F16, name=f"m48b_{k}",
                                    tag="m48b")
                    lo_b = dig[:, 0, :, None].broadcast_to([P, TT, 16])
                    hi_b = dig[:, 1, :, None].broadcast_to([P, TT, NC16])
                    C_b = c16s[k][:, :, None].broadcast_to([P, TT, NC16])
                    nc.vector.tensor_tensor(s16m, i16r, lo_b, ALU.is_equal)
                    # eq48 = (iota48 == rank//16); m48c = eq48*C (separate
                    # tile), then m48t *= tokid in place
                    nc.vector.tensor_tensor(m48t, i48r, hi_b, ALU.is_equal)
                    nc.vector.tensor_tensor(m48c, m48t, C_b, ALU.mult)
                    nc.vector.tensor_tensor(m48t, m48t, tokr, ALU.mult)
                    ids_t = zc.tile([P, NC16], F32, name="ids_t",
                                    tag="pz2")[0:16, :]
                    w_t = zc.tile([P, NC16], F32, name="w_t",
                                  tag="pz3")[0:16, :]
                    for tt in range(TT):
                        nc.tensor.matmul(ids_t, s16m[:, tt, :],
                                         m48t[:, tt, :],
                                         start=(tt == 0), stop=(tt == TT - 1))
                        nc.tensor.matmul(w_t, s16m[:, tt, :],
                                         m48c[:, tt, :],
                                         start=(tt == 0), stop=(tt == TT - 1))
                    ids_f = cmp1.tile([16, NC16], F32, name="ids_f", tag="ids_f")
                    nc.vector.tensor_copy(ids_f, ids_t)
                    nc.vector.tensor_copy(W16[k], w_t)
                    bc_ps = zc.tile([P, NC16], F32, name="bc_ps", tag="pz3")
                    nc.tensor.matmul(bc_ps, BI, ids_f, start=True, stop=True)
                    nc.vector.tensor_copy(idx16[k], bc_ps)
                    # gathers for this expert start as soon as idx is ready
                    halves = []
                    for hh, hcap in enumerate(_halves(CAPS[k])):
                        base = hh * ACH
                        xgh = ag.tile([P, KT_H, hcap], BF16, name=f"xg{k}_{hh}",
                                      tag="xg", bufs=2)
                        csl = slice(base // 16, (base + hcap) // 16)
                        nc.gpsimd.dma_gather(
                            xgh, xbf_d.ap(), idx16[k][:, csl],
                            hcap, hcap, H, transpose=True)
                        halves.append(xgh)
                    return halves

                def shared_c_group(hc, s, gi):
                    sw2q = sw2q_t[hc]
                    hsl = slice(hc * 512, (hc + 1) * 512)
                    ssl = slice(s * P, (s + 1) * P)
                    pz = zc.tile([P, 512], F32, name="spz", tag=f"pz{gi % 2}")
                    for ki in range(KT_I):
                        nc.tensor.matmul(pz, ys[:, ki, ssl], sw2q[:, ki, :],
                                         start=(ki == 0), stop=(ki == KT_I - 1))
                    ot = so.tile([P, 512], BF16, name="ot", tag="ot")
                    nc.vector.tensor_copy(ot, pz)
                    nc.scalar.dma_start(out=out_d.ap()[ssl, hsl], in_=ot)

                # PE fill order across the compaction window:
                xg = [None, None]
                xg[0] = compact_mms(0)
                xg[1] = compact_mms(1)
                shared_a_block(N_SA_ROUTER + 1, use_sigmoid=False)
                gi = 0
                for hc in range(4):
                    for s in range(TS // P):
                        shared_c_group(hc, s, gi)
                        gi += 1
                # exports on the scalar HWDGE queue: keeps the Q7 queue
                # free for the critical-path gathers
                for k in range(E_PER_CORE):
                    nc.sync.dma_start(out=ids_d.ap()[k], in_=idx16[k][0:16, :])
                    # W16[q, 8s + r] -> W128[r*16 + q, s]
                    for r in range(8):
                        nc.sync.dma_start(out=W128[k][16 * r:16 * (r + 1), :],
                                          in_=W16[k][:, r::8])

                # ---------------- routed FFN per expert ----------------
                y = [sres.tile([P, KT_I, capc[k]], BF16, name=f"y{k}",
                               tag="y", bufs=1)
                     for k in range(E_PER_CORE)]
                for k in range(E_PER_CORE):
                    # pass A: y = silu(x@w1T) * (x@w3T) over COMPUTE cap
                    cchunks = [ACH, capc[k] - ACH]
                    for c, hcap in enumerate(cchunks):
                        for h in range(2):
                            w1h, w3h = w1h_t[(k, h)], w3h_t[(k, h)]
                            for m in range(4):
                                mi = h * 4 + m
                                msl = slice(m * P, (m + 1) * P)
                                pg = aps.tile([P, ACH], F32, name="pg",
                                              tag=f"pg{m % 2}")[:, :hcap]
                                pu = aps.tile([P, ACH], F32, name="pu",
                                              tag=f"pu{m % 2}")[:, :hcap]
                                for kt in range(KT_H):
                                    nc.tensor.matmul(
                                        pg, w1h[:, kt, msl],
                                        xg[k][c][:, kt, :hcap],
                                        start=(kt == 0), stop=(kt == KT_H - 1))
                                for kt in range(KT_H):
                                    nc.tensor.matmul(
                                        pu, w3h[:, kt, msl],
                                        xg[k][c][:, kt, :hcap],
                                        start=(kt == 0), stop=(kt == KT_H - 1))
                                sg = ay.tile([P, ACH], BF16, name="sg",
                                             tag="sg")[:, :hcap]
                                nc.scalar.activation(sg, pg, AF.Silu)
                                csl = slice(c * ACH, c * ACH + hcap)
                                nc.vector.tensor_tensor(y[k][:, mi, csl], sg,
                                                        pu, ALU.mult)
                    # pass C: z = W * (y @ w2T); groups cycle 4 psum banks
                    gi = 0
                    ns_full, rem = divmod(capc[k], P)
                    sdims = [P] * ns_full + ([rem] if rem else [])
                    for q in range(4):
                        w2q = w2h_t[(k, q)]
                        hsl = slice(q * 512, (q + 1) * 512)
                        for s, sdim in enumerate(sdims):
                            ssl = slice(s * P, s * P + sdim)
                            pz = zc.tile([P, 512], F32, name="pz",
                                         tag=f"pz{gi % 4}")[:sdim, :]
                            gi += 1
                            for ki in range(KT_I):
                                nc.tensor.matmul(pz, y[k][:, ki, ssl],
                                                 w2q[:, ki, :],
                                                 start=(ki == 0),
                                                 stop=(ki == KT_I - 1))
                            zt = zo.tile([P, 512], BF16, name="zc",
                                         tag="zc")[:sdim, :]
                            nc.vector.tensor_scalar_mul(zt, pz,
                                                        W128[k][:sdim, s:s + 1])
                            nc.sync.dma_start(out=z_d.ap()[k, ssl, hsl],
                                               in_=zt)

            es_.close()

    nc.compile()
    return nc


_NC_CACHE = {}


def _get_nc(capc):
    if capc not in _NC_CACHE:
        _NC_CACHE[capc] = build_nc(capc)
    return _NC_CACHE[capc]


def _route_counts(x, gate_w, expert_bias):
    """Host-side routing counts, used ONLY for load-balanced expert->core
    assignment and compute-capacity sizing (sharding decisions); the
    device recomputes routing."""
    logits = x @ gate_w.T
    scores = 1.0 / (1.0 + np.exp(-logits))
    sel = scores + expert_bias[None, :]
    grp = sel.reshape(T, G, E // G)
    t2 = np.sort(grp, -1)[:, :, -2:].sum(-1)
    gidx = np.argsort(t2, -1)[:, -2:]
    gmask = np.zeros((T, G), bool)
    gmask[np.arange(T)[:, None], gidx] = True
    emask = np.repeat(gmask, E // G, axis=1)
    masked = np.where(emask, sel, -np.inf)
    ids = np.argsort(masked, -1)[:, -K_TOP:]
    return np.bincount(ids.ravel(), minlength=E)


def _pc(aT, ncol):
    """[KT*128, C] -> [C//ncol, 128, KT*ncol] partition-contiguous."""
    kt = aT.shape[0] // P
    nch = aT.shape[1] // ncol
    a = aT.reshape(kt, P, nch, ncol)
    return np.ascontiguousarray(
        np.transpose(a, (2, 1, 0, 3)).reshape(nch, P, kt * ncol))


def kernel(hidden_states, gate_w, expert_bias, w1, w3, w2, sw1, sw3, sw2):
    x = np.ascontiguousarray(hidden_states, dtype=np.float32)
    bf = ml_dtypes.bfloat16
    xhi = x.astype(bf)
    xlo = (x - xhi.astype(np.float32)).astype(bf)
    gw = np.ascontiguousarray(gate_w.astype(np.float32))
    ghi = gw.astype(bf)
    glo = (gw - ghi.astype(np.float32)).astype(bf)
    xbf = np.ascontiguousarray(xhi)
    xhiT = np.ascontiguousarray(xhi.T)
    xloT = np.ascontiguousarray(xlo.T)
    bias = expert_bias.astype(np.float32)
    biasb = np.ascontiguousarray(np.broadcast_to(bias[None, :], (P, E)))

    # partition-contiguous streaming layouts
    xhi_r = _pc(xhiT, TCH)                      # [NTOK, P, KT_H*TCH]
    xlo_r = _pc(xloT, TCH)
    gpair = np.concatenate([np.ascontiguousarray(ghi.T),
                            np.zeros((H, E), ghi.dtype),
                            np.ascontiguousarray(glo.T)], axis=1)
    g_r = _pc(np.ascontiguousarray(gpair), 3 * E)[0]
    sw13_r = np.stack([_pc(np.ascontiguousarray(sw1.T.astype(bf)), 256),
                       _pc(np.ascontiguousarray(sw3.T.astype(bf)), 256)])
    sw2_r = _pc(np.ascontiguousarray(sw2.T.astype(bf)), 512)

    # load-balanced assignment: pair i-th largest with i-th smallest
    counts = _route_counts(x.astype(np.float64), gw.astype(np.float64),
                           bias.astype(np.float64))
    order = np.argsort(-counts)
    assign = [(int(order[i]), int(order[E - 1 - i])) for i in range(N_CORES)]
    # compute capacity: actual max per slot + margin, rounded to 8
    cnt0 = max(counts[a] for a, _ in assign)
    cnt1 = max(counts[b] for _, b in assign)
    capc = (min(CAPS[0], max(ACH + 8, -(-(int(cnt0) + 8) // 8) * 8)),
            min(CAPS[1], max(ACH + 8, -(-(int(cnt1) + 8) // 8) * 8)))

    w1tb = np.transpose(w1, (0, 2, 1)).astype(bf)
    w3tb = np.transpose(w3, (0, 2, 1)).astype(bf)
    w2tb = np.transpose(w2, (0, 2, 1)).astype(bf)

    in_maps = []
    for c in range(N_CORES):
        e_hi, e_lo = assign[c]
        esel = np.zeros((P, 2, E), np.float32)
        esel[:, 0, e_hi] = 1.0
        esel[:, 1, e_lo] = 1.0
        pick = [e_hi, e_lo]
        w13_r = np.stack([
            np.stack([
                np.stack([_pc(np.ascontiguousarray(w1tb[e]), 512),
                          _pc(np.ascontiguousarray(w3tb[e]), 512)], axis=1)[h]
                for h in range(2)])
            for e in pick])                     # [2, 2, 2, P, KT_H*512]
        w2_r = np.stack([_pc(np.ascontiguousarray(w2tb[e]), 512)
                         for e in pick])        # [2, 4, P, KT_I*512]
        xbs_r = _pc(np.ascontiguousarray(xhiT[:, TS * c:TS * (c + 1)]), TS)[0]
        in_maps.append({
            "xhi": xhi_r,
            "xlo": xlo_r,
            "xbf": xbf,
            "g": g_r,
            "biasb": biasb,
            "esel": esel,
            "w13": np.ascontiguousarray(w13_r),
            "w2": np.ascontiguousarray(w2_r),
            "sw13": sw13_r,
            "sw2": sw2_r,
            "xbs": xbs_r,
        })

    nc = _get_nc(capc)
    res = run_bass_kernel_spmd(nc, in_maps, list(range(N_CORES)))

    out = np.zeros((T, H), np.float32)
    for c in range(N_CORES):
        r = res.results[c]
        z = np.asarray(r["z"], dtype=np.float32)          # [2, CAPS[0], H]
        ids = np.asarray(r["ids"], dtype=np.int64)        # [2, 16, NC16]
        for k in range(E_PER_CORE):
            slot_ids = ids[k].T.reshape(-1)               # slot i at [i%16, i//16]
            nz = np.nonzero(slot_ids)[0]
            cnt = (nz[-1] + 1) if len(nz) else 0
            cnt = min(cnt, capc[k])
            if cnt:
                out[slot_ids[:cnt]] += z[k, :cnt]
        out[TS * c:TS * (c + 1)] += np.asarray(r["out"], dtype=np.float32)
    kernel.last_result = res
    return out


# revision 42
# speedup vs baseline: 1.1173x; 1.0144x over previous
"""MoE (BailingMoeV2.5) Trainium2 kernel — 8-core expert-parallel, SPARSE.

T=2048 tokens, H=2048 hidden, E=16 experts (4 groups, top-2 groups,
top-4 experts), I=1024 expert intermediate, shared expert IS=1024,
routed scale 2.5.

Each core owns 2 experts (host pairs high-count with low-count experts;
gather slot capacities 768/640, compute capacities trimmed to the
actual max routed counts + margin, rounded to 8):
  1. Router: logits via lossless-ish bf16 hi/lo split (3 bf16 passes),
     sigmoid scores, batched grouped top-k epilogue (3 pieces: 8/4/4
     token-tiles, overlapped with the score stream) -> per-token
     combine weights C2[token, 2] for this core's experts (x2.5,
     renormalized).
  2. Device-side stream compaction per expert (cumsum-matmul rank +
     batched one-hot mask build via big tensor_tensor ops + fused
     onehot matmuls) -> token-id list (int16, dma_gather layout) +
     per-slot weights.  Padding slots gather token 0 with W=0.
  3. dma_gather (transpose mode) pulls selected tokens from the bf16
     token-major x into feature-major [128, 16, cap].
  4. bf16 SwiGLU FFN per expert over the COMPUTE capacity only;
     output scaled by W -> z + ids exported.
  5. Shared expert (bf16) on the core's 256-token slice; shared-A
     blocks 0-4 interleave with router chunks (silu computed as
     x*sigmoid(x) to avoid ACT-table thrash), blocks 5-7 + shared
     pass C fill the compaction window.
Host unshard: out[ids] += z per (core, slot); out[slice_c] += shared_c.

DMA queues: sync(SP) = xhi router chunks then routed weights;
scalar(Act) = chunk-0 split + shared-expert feeds then outputs;
gpsimd = xlo router chunks, idx bookkeeping + gathers.
All large streams use host-side partition-contiguous layouts
(one >=4KB contiguous run per partition per DMA).
"""
import sys
from contextlib import ExitStack

sys.path.insert(0, "/opt/trn_rl_repo")

import numpy as np
import ml_dtypes

import concourse.bass as bass
import concourse.mybir as mybir
import concourse.tile as tile
from concourse import bacc
from concourse.bass_utils import run_bass_kernel_spmd
from concourse.masks import make_identity, make_upper_triangular

P = 128
T, H, E, K_TOP, I = 2048, 2048, 16, 4, 1024
G = 4
IS = 1024
N_CORES = 8
E_PER_CORE = E // N_CORES  # 2
TS = T // N_CORES          # 256
ROUTED_SCALE = 2.5

KT_H = H // P              # 16
KT_I = I // P              # 8
NTOK = 8                   # router token chunks of 256
TCH = T // NTOK            # 256
TT = T // P                # 16
CAPS = (768, 640)          # gather slot capacity (multiple of 128)
NC16 = 48                  # idx cols allocated (CAPS[0]/16)
ACH = 384                  # pass-A slot chunk (psum bank fits 384 fp32)
N_SA_ROUTER = 6            # shared-A blocks interleaved with router

F32 = mybir.dt.float32
F16 = mybir.dt.float16
BF16 = mybir.dt.bfloat16
I16 = mybir.dt.int16
I32 = mybir.dt.int32
AX = mybir.AxisListType.X
ALU = mybir.AluOpType
AF = mybir.ActivationFunctionType


def _halves(cap):
    return (ACH, cap - ACH)


def build_nc(capc):
    """capc: per-slot COMPUTE capacity (<= CAPS, multiple of 8)."""
    nc = bacc.Bacc(None, target_bir_lowering=False, debug=False)

    # all streaming inputs are partition-contiguous: [.., P, inner]
    xhi_d = nc.declare_dram_parameter("xhi", [NTOK, P, KT_H * TCH], BF16, isOutput=False)
    xlo_d = nc.declare_dram_parameter("xlo", [NTOK, P, KT_H * TCH], BF16, isOutput=False)
    xbf_d = nc.declare_dram_parameter("xbf", [T, H], BF16, isOutput=False)
    g_d = nc.declare_dram_parameter("g", [P, KT_H * 3 * E], BF16, isOutput=False)
    biasb_d = nc.declare_dram_parameter("biasb", [P, E], F32, isOutput=False)
    esel_d = nc.declare_dram_parameter("esel", [P, 2, E], F32, isOutput=False)
    w13_d = nc.declare_dram_parameter("w13", [E_PER_CORE, 2, 2, P, KT_H * 512], BF16, isOutput=False)
    w2_d = nc.declare_dram_parameter("w2", [E_PER_CORE, 4, P, KT_I * 512], BF16, isOutput=False)
    sw13_d = nc.declare_dram_parameter("sw13", [2, 4, P, KT_H * 256], BF16, isOutput=False)
    sw2_d = nc.declare_dram_parameter("sw2", [4, P, KT_I * 512], BF16, isOutput=False)
    xbs_d = nc.declare_dram_parameter("xbs", [P, KT_H * TS], BF16, isOutput=False)

    z_d = nc.declare_dram_parameter("z", [E_PER_CORE, CAPS[0], H], BF16, isOutput=True)
    ids_d = nc.declare_dram_parameter("ids", [E_PER_CORE, 16, NC16], I16, isOutput=True)
    out_d = nc.declare_dram_parameter("out", [TS, H], BF16, isOutput=True)

    def _c3(ap, n):
        return ap.rearrange("p (kt n) -> p kt n", n=n)

    with tile.TileContext(nc) as tc:
        with tc.tile_pool(name="res", bufs=1) as res:
            # ---------------- persistent small tiles ----------------
            sc_all = res.tile([P, TT, E], F32, name="sc_all")
            C2_sb = res.tile([P, TT, E_PER_CORE], F32, name="C2_sb")
            M2_sb = res.tile([P, TT, E_PER_CORE], F32, name="M2_sb")
            ident = res.tile([P, P], F32, name="ident")
            make_identity(nc, ident)
            tril = res.tile([P, P], F32, name="tril")
            make_upper_triangular(nc, tril, val=1.0, diag=True)
            ones128p = res.tile([P, 1], F32, name="ones128p")
            nc.vector.memset(ones128p, 1.0)
            ones_row = res.tile([1, P], F32, name="ones_row")
            nc.vector.memset(ones_row, 1.0)
            iotas = res.tile([P, 80], F32, name="iotas")
            iota16 = iotas[:, 0:16]
            iota48 = iotas[:, 16:64]
            tokid = iotas[:, 64:80]
            ii = res.tile([P, NC16], I32, name="ii")
            nc.gpsimd.iota(ii[:, 0:16], pattern=[[1, 16]], base=0, channel_multiplier=0)
            nc.vector.tensor_copy(iota16, ii[:, 0:16])
            nc.gpsimd.iota(ii[:, 0:NC16], pattern=[[1, NC16]], base=0, channel_multiplier=0)
            nc.vector.tensor_copy(iota48, ii[:, 0:NC16])
            nc.gpsimd.iota(ii[:, 0:TT], pattern=[[P, TT]], base=0, channel_multiplier=1)
            nc.vector.tensor_copy(tokid, ii[:, 0:TT])

            idx16 = [res.tile([P, NC16], I16, name=f"idx16_{k}")
                     for k in range(E_PER_CORE)]
            W128 = [res.tile([P, 6], F32, name=f"W128_{k}")
                    for k in range(E_PER_CORE)]
            W16 = [res.tile([16, NC16], F32, name=f"W16_{k}")
                   for k in range(E_PER_CORE)]
            # iota/token-id repeats for the batched compaction masks,
            # built up-front while the engines are otherwise idle
            i16r = res.tile([P, TT, 16], F16, name="i16r")
            nc.vector.tensor_copy(
                i16r, iota16[:, None, :].broadcast_to([P, TT, 16]))
            i48r = res.tile([P, TT, NC16], F16, name="i48r")
            nc.vector.tensor_copy(
                i48r, iota48[:, None, :].broadcast_to([P, TT, NC16]))
            tokr = res.tile([P, TT, NC16], F16, name="tokr")
            nc.vector.tensor_copy(
                tokr, tokid[:, :, None].broadcast_to([P, TT, NC16]))
            # block-identity BI[q, p] = (p % 16 == q), for idx broadcast
            BI = res.tile([16, P], F32, name="BI")
            bii = res.tile([16, P], I32, name="bii")
            nc.gpsimd.iota(bii, pattern=[[1, P]], base=0, channel_multiplier=0)
            nc.vector.tensor_scalar(bii, bii, 15, None, ALU.bitwise_and)
            bif = res.tile([16, P], F32, name="bif")
            nc.vector.tensor_copy(bif, bii)
            qcolf = res.tile([16, 1], F32, name="qcolf")
            qcol = res.tile([16, 1], I32, name="qcol")
            nc.gpsimd.iota(qcol, pattern=[[1, 1]], base=0, channel_multiplier=1)
            nc.vector.tensor_copy(qcolf, qcol)
            nc.vector.tensor_scalar(BI, bif, qcolf, None, ALU.is_equal)

            # shared-expert pools at outer scope
            es_ = ExitStack()
            swp = es_.enter_context(tc.tile_pool(name="sw", bufs=3))
            sres = es_.enter_context(tc.tile_pool(name="sres", bufs=1))
            so = es_.enter_context(tc.tile_pool(name="so", bufs=2))
            aps = es_.enter_context(tc.tile_pool(name="aps", bufs=1, space="PSUM"))
            # scalar (Act) HWDGE queue: chunk-0 xhi halves first (the
            # sync queue starts with g weights + chunks 1-7), then the
            # shared-expert feeds.
            es0 = ExitStack()
            rx0 = es0.enter_context(tc.tile_pool(name="rx0", bufs=1))
            xh0a = rx0.tile([P, KT_H // 2, TCH], BF16, name="xh0a")
            xh0b = rx0.tile([P, KT_H // 2, TCH], BF16, name="xh0b")
            xl0a = rx0.tile([P, KT_H // 2, TCH], BF16, name="xl0a")
            xl0b = rx0.tile([P, KT_H // 2, TCH], BF16, name="xl0b")
            nc.scalar.dma_start(out=xh0a, in_=_c3(xhi_d.ap()[0], TCH)[:, 0:KT_H // 2, :])
            nc.gpsimd.dma_start(out=xl0a, in_=_c3(xlo_d.ap()[0], TCH)[:, 0:KT_H // 2, :])
            nc.scalar.dma_start(out=xl0b, in_=_c3(xlo_d.ap()[0], TCH)[:, KT_H // 2:, :])
            # shared expert feeds on scalar queue
            xs = sres.tile([P, KT_H, TS], BF16, name="xs")
            nc.scalar.dma_start(out=xs, in_=_c3(xbs_d.ap(), TS))
            # sw13 loads self-throttle via the swx slot rotation (later
            # tiles wait on earlier shared-A blocks); sw2 queues behind
            # them so its 4MB stays out of the router-chunk congestion
            # window but lands before shared-C needs it.
            sw1q_t, sw3q_t, sw2q_t = {}, {}, {}
            for q in range(4):
                sw1q_t[q] = swp.tile([P, KT_H, 256], BF16, name="sw1q",
                                     tag="swx", bufs=2)
                sw3q_t[q] = swp.tile([P, KT_H, 256], BF16, name="sw3q",
                                     tag="swx", bufs=2)
                nc.scalar.dma_start(out=sw1q_t[q], in_=_c3(sw13_d.ap()[0, q], 256))
                nc.scalar.dma_start(out=sw3q_t[q], in_=_c3(sw13_d.ap()[1, q], 256))
            for q in range(4):
                sw2q_t[q] = swp.tile([P, KT_I, 512], BF16, name="sw2q",
                                     tag="sw2", bufs=4)
                nc.scalar.dma_start(out=sw2q_t[q], in_=_c3(sw2_d.ap()[q], 512))
            ys = sres.tile([P, KT_I, TS], BF16, name="ys")

            def shared_a_block(mi, use_sigmoid):
                h, m = mi // 2, mi % 2
                sw1h, sw3h = sw1q_t[h], sw3q_t[h]
                msl = slice(m * P, (m + 1) * P)
                pg = aps.tile([P, ACH], F32, name="spg",
                              tag=f"pg{mi % 2}")[:, :TS]
                pu = aps.tile([P, ACH], F32, name="spu",
                              tag=f"pu{mi % 2}")[:, :TS]
                for kt in range(KT_H):
                    nc.tensor.matmul(pg, sw1h[:, kt, msl], xs[:, kt, :],
                                     start=(kt == 0), stop=(kt == KT_H - 1))
                for kt in range(KT_H):
                    nc.tensor.matmul(pu, sw3h[:, kt, msl], xs[:, kt, :],
                                     start=(kt == 0), stop=(kt == KT_H - 1))
                sg = so.tile([P, TS], BF16, name="ssg", tag="ssg")
                if use_sigmoid:
                    # silu(x) = x * sigmoid(x): avoids Sigmoid<->Silu
                    # ACT-table reloads between router chunks
                    nc.scalar.activation(sg, pg, AF.Sigmoid)
                    st = so.tile([P, TS], BF16, name="sst", tag="sst")
                    nc.vector.tensor_tensor(st, sg, pu, ALU.mult)
                    nc.vector.tensor_tensor(ys[:, mi, :], st, pg, ALU.mult)
                else:
                    nc.scalar.activation(sg, pg, AF.Silu)
                    nc.vector.tensor_tensor(ys[:, mi, :], sg, pu, ALU.mult)

            # =================== router (bf16 hi/lo) ===================
            with tc.tile_pool(name="rt", bufs=2) as rt, \
                 tc.tile_pool(name="rt1", bufs=1) as rt1, \
                 tc.tile_pool(name="rxn", bufs=2) as rxn, \
                 tc.tile_pool(name="rtp", bufs=2, space="PSUM") as rtp:
                # gcat[:, kt, 0:16] = ghi, [:, kt, 32:48] = glo (16:32
                # zero pad): one M=48 stationary pass computes ghi@xh and
                # glo@xh together; the pad keeps glo's psum rows at base
                # partition 32 (engine partition-offset constraint)
                gcat = rt1.tile([P, KT_H, 3 * E], BF16, name="gcat")
                nc.sync.dma_start(out=gcat, in_=_c3(g_d.ap(), 3 * E))
                nc.sync.dma_start(out=xh0b,
                                  in_=_c3(xhi_d.ap()[0], TCH)[:, KT_H // 2:, :])
                biasb = rt1.tile([P, E], F32, name="biasb")
                nc.sync.dma_start(out=biasb, in_=biasb_d.ap())
                esel = rt1.tile([P, 2, E], F32, name="esel")
                nc.sync.dma_start(out=esel, in_=esel_d.ap())
                sT = rt1.tile([16, T], F32, name="sT")

                def epilogue_part(ts0, nts):
                    """Grouped top-k for tt in [ts0, ts0+nts) -> C2/M2."""
                    tsl = slice(ts0, ts0 + nts)
                    sc = sc_all[:, tsl, :]
                    selA = rt.tile([P, 8, E], F32, name="selA",
                                   tag="selA")[:, :nts, :]
                    nc.vector.tensor_tensor(
                        selA, sc,
                        biasb[:, None, :].broadcast_to([P, nts, E]), ALU.add)
                    a = selA[:, :, 0::4]
                    b = selA[:, :, 1::4]
                    c_ = selA[:, :, 2::4]
                    d = selA[:, :, 3::4]
                    t4 = rt.tile([P, 8, 6, G], F32, name="t4",
                                 tag="t4")[:, :nts, :, :]
                    m1, n1, m2, n2, gs, tmp = (t4[:, :, j, :] for j in range(6))
                    nc.vector.tensor_tensor(m1, a, b, ALU.max)
                    nc.vector.tensor_tensor(n1, a, b, ALU.min)
                    nc.vector.tensor_tensor(m2, c_, d, ALU.max)
                    nc.vector.tensor_tensor(n2, c_, d, ALU.min)
                    nc.vector.tensor_tensor(gs, m1, m2, ALU.add)
                    nc.vector.tensor_tensor(tmp, m1, n1, ALU.add)
                    nc.vector.tensor_tensor(gs, gs, tmp, ALU.max)
                    nc.vector.tensor_tensor(tmp, m2, n2, ALU.add)
                    nc.vector.tensor_tensor(gs, gs, tmp, ALU.max)
                    g2 = rt.tile([P, 8, 6], F32, name="g2",
                                 tag="g2")[:, :nts, :]
                    ga, gb = gs[:, :, 0::2], gs[:, :, 1::2]
                    gmx, gmn = g2[:, :, 0:2], g2[:, :, 2:4]
                    gthr = g2[:, :, 4:5]
                    gt2 = g2[:, :, 5:6]
                    nc.vector.tensor_tensor(gmx, ga, gb, ALU.max)
                    nc.vector.tensor_tensor(gmn, ga, gb, ALU.min)
                    nc.vector.tensor_tensor(gthr, gmx[:, :, 0:1], gmx[:, :, 1:2],
                                            ALU.min)
                    nc.vector.tensor_tensor(gt2, gmn[:, :, 0:1], gmn[:, :, 1:2],
                                            ALU.max)
                    nc.vector.tensor_tensor(gthr, gthr, gt2, ALU.max)
                    gmask = rt.tile([P, 8, G], F32, name="gmask",
                                    tag="gmask")[:, :nts, :]
                    nc.vector.tensor_tensor(
                        gmask, gs, gthr.broadcast_to([P, nts, G]), ALU.is_ge)
                    emask = rt.tile([P, 8, E], F32, name="emask",
                                    tag="emask")[:, :nts, :]
                    for j in range(4):
                        nc.vector.tensor_copy(emask[:, :, j::4], gmask)
                    masked = rt.tile([P, 8, E], F32, name="masked",
                                     tag="masked")[:, :nts, :]
                    nc.vector.tensor_scalar_add(emask, emask, -1.0)
                    nc.vector.scalar_tensor_tensor(masked, emask, 1e30, selA,
                                                   ALU.mult, ALU.add)
                    m8s = rt.tile([P, 8, 8], F32, name="m8s",
                                  tag="m8s")[:, :nts, :]
                    for tt in range(nts):
                        nc.vector.max(m8s[:, tt, :], masked[:, tt, :])
                    selm = rt.tile([P, 8, E], F32, name="selm",
                                   tag="selm")[:, :nts, :]
                    nc.vector.tensor_tensor(
                        selm, masked,
                        m8s[:, :, 3:4].broadcast_to([P, nts, E]), ALU.is_ge)
                    cw = rt.tile([P, 8, E], F32, name="cw",
                                 tag="cw")[:, :nts, :]
                    nc.vector.tensor_tensor(cw, sc, selm, ALU.mult)
                    den = rt.tile([P, 8, 2], F32, name="den",
                                  tag="den")[:, :nts, :]
                    nc.vector.reduce_sum(den[:, :, 0:1], cw, AX)
                    nc.vector.tensor_scalar_add(den[:, :, 0:1], den[:, :, 0:1],
                                                1e-20)
                    nc.vector.reciprocal(den[:, :, 1:2], den[:, :, 0:1])
                    nc.vector.tensor_scalar_mul(den[:, :, 1:2], den[:, :, 1:2],
                                                ROUTED_SCALE)
                    nc.vector.tensor_tensor(
                        cw, cw, den[:, :, 1:2].broadcast_to([P, nts, E]), ALU.mult)
                    esm = rt.tile([P, 8, E], F32, name="esm",
                                  tag="esm")[:, :nts, :]
                    for k in range(E_PER_CORE):
                        nc.vector.tensor_tensor(
                            esm, cw,
                            esel[:, k, :][:, None, :].broadcast_to([P, nts, E]),
                            ALU.mult)
                        nc.vector.reduce_sum(C2_sb[:, tsl, k:k + 1], esm, AX)
                    nc.vector.tensor_scalar(
                        M2_sb[:, tsl, :].rearrange("p a b -> p (a b)"),
                        C2_sb[:, tsl, :].rearrange("p a b -> p (a b)"),
                        0.0, None, ALU.is_gt)

                for n in range(NTOK):
                    if n == 0:
                        xh_parts = [(xh0a, 0), (xh0b, KT_H // 2)]
                        xl_parts = [(xl0a, 0), (xl0b, KT_H // 2)]
                    else:
                        xh = rxn.tile([P, KT_H, TCH], BF16, name="xh",
                                      tag="xh", bufs=2)
                        xl = rxn.tile([P, KT_H, TCH], BF16, name="xl",
                                      tag="xl", bufs=1)
                        nc.sync.dma_start(out=xh, in_=_c3(xhi_d.ap()[n], TCH))
                        nc.gpsimd.dma_start(out=xl, in_=_c3(xlo_d.ap()[n], TCH))
                        xh_parts = [(xh, 0)]
                        xl_parts = [(xl, 0)]
                    tksl = slice(n * TCH, (n + 1) * TCH)
                    ps = rtp.tile([48, TCH], F32, name="ps_r", tag="ps_r")
                    # pass 1: [ghi|glo] @ xh -> rows 0:32; pass 2:
                    # ghi @ xl accumulates into rows 0:16
                    for pi, (x_, koff) in enumerate(xh_parts):
                        nkt = x_.shape[1]
                        for kt in range(nkt):
                            nc.tensor.matmul(
                                ps, gcat[:, koff + kt, :], x_[:, kt, :],
                                start=(pi == 0 and kt == 0), stop=False)
                    nl = len(xl_parts)
                    for pi, (x_, koff) in enumerate(xl_parts):
                        nkt = x_.shape[1]
                        for kt in range(nkt):
                            nc.tensor.matmul(
                                ps[0:16, :], gcat[:, koff + kt, 0:E],
                                x_[:, kt, :],
                                start=False,
                                stop=(pi == nl - 1 and kt == nkt - 1))
                    s2 = rt.tile([16, 2, TCH], F32, name="s2", tag="s2")
                    nc.vector.tensor_copy(s2[:, 1, :], ps[32:48, :])
                    nc.vector.tensor_tensor(s2[:, 0, :], ps[0:16, :],
                                            s2[:, 1, :], ALU.add)
                    nc.scalar.activation(sT[:, tksl], s2[:, 0, :], AF.Sigmoid)
                    for tt in range(2 * n, 2 * n + 2):
                        pst = rtp.tile([P, 16], F32, name="pst", tag="pst")
                        nc.tensor.transpose(pst, sT[:, tt * P:(tt + 1) * P],
                                            ident[:16, :16])
                        nc.vector.tensor_copy(sc_all[:, tt, :], pst)
                    if n < N_SA_ROUTER:
                        shared_a_block(n, use_sigmoid=True)
                    if n == 3:
                        epilogue_part(0, 8)
                    elif n == 5:
                        epilogue_part(8, 4)
                epilogue_part(12, 4)
            es0.close()   # free chunk-0 tiles before FFN pools allocate

            # ============ compaction + shared + routed FFN ============
            # PSUM banks (8): aps 4 (pg0,pg1,pu0,pu1; shared-A + routed A),
            # zc 4 (pz0..pz3): shared-C on pz0/pz1, compaction accum on
            # pz2/pz3, routed C cycles all four.
            with tc.tile_pool(name="cmp", bufs=1) as cmp, \
                 tc.tile_pool(name="cmp1", bufs=2) as cmp1, \
                 tc.tile_pool(name="zc", bufs=1, space="PSUM") as zc, \
                 tc.tile_pool(name="aw", bufs=4) as aw, \
                 tc.tile_pool(name="w2p", bufs=2) as w2p, \
                 tc.tile_pool(name="ay", bufs=2) as ay, \
                 tc.tile_pool(name="ag", bufs=2) as ag, \
                 tc.tile_pool(name="zo", bufs=2) as zo:

                # sync (SP) HWDGE queue (behind router xhi stream):
                # routed weights, ordered by first need
                w1h_t, w3h_t, w2h_t = {}, {}, {}

                def _w13(k, h):
                    w1h = aw.tile([P, KT_H, 512], BF16, name="w1h", tag="wA")
                    w3h = aw.tile([P, KT_H, 512], BF16, name="w3h", tag="wA")
                    nc.sync.dma_start(out=w1h, in_=_c3(w13_d.ap()[k, h, 0], 512))
                    nc.sync.dma_start(out=w3h, in_=_c3(w13_d.ap()[k, h, 1], 512))
                    w1h_t[(k, h)] = w1h
                    w3h_t[(k, h)] = w3h

                def _w2(k, q):
                    w2q = w2p.tile([P, KT_I, 512], BF16, name="w2q", tag="w2")
                    nc.sync.dma_start(out=w2q, in_=_c3(w2_d.ap()[k, q], 512))
                    w2h_t[(k, q)] = w2q

                _w13(0, 0)
                _w13(0, 1)
                _w2(0, 0)
                _w2(0, 1)
                _w13(1, 0)
                _w13(1, 1)
                _w2(0, 2)
                _w2(0, 3)
                for q in range(4):
                    _w2(1, q)

                # ---- compaction: rank chains + batched mask build ----
                # phase a (both experts): rank via cumsum matmuls + scan
                # + digit split; then per expert: one-hot masks for ALL
                # 16 token tiles in a few large vector ops (fp16),
                # scatter matmuls, idx broadcast, gathers.  shared-A
                # block 6 leads so the PE has work while the vector
                # engine runs the final epilogue piece.
                shared_a_block(N_SA_ROUTER, use_sigmoid=False)
                digs, c16s = [], []
                for k in range(E_PER_CORE):
                    M = M2_sb[:, :, k]
                    cum_t = zc.tile([P, NC16], F32, name="cum_t",
                                    tag="pz2")[:, 0:TT]
                    cmt = zc.tile([P, NC16], F32, name="cmt", tag="pz3")
                    tot_ps = cmt[0:1, 0:TT]
                    nc.tensor.matmul(cum_t, tril, M, start=True, stop=True)
                    nc.tensor.matmul(tot_ps, ones128p, M, start=True, stop=True)
                    tot = cmp1.tile([1, 3, TT], F32, name="tot", tag="tot")
                    ex0, ex1 = tot[:, 1, :], tot[:, 2, :]
                    nc.vector.tensor_copy(tot[:, 0, :], tot_ps)
                    nc.vector.memset(ex0[:, 0:1], 0.0)
                    nc.vector.tensor_copy(ex0[:, 1:], tot[:, 0, 0:TT - 1])
                    nc.vector.tensor_tensor_scan(ex1, ex0, ex0, 0.0,
                                                 ALU.add, ALU.bypass)
                    carry_ps = cmt[:, TT:2 * TT]
                    nc.tensor.matmul(carry_ps, ones_row, ex1, start=True, stop=True)
                    # rank, with non-routed tokens pushed out of range
                    # (+2048: keeps rank%16, sends rank//16 beyond 47, so
                    # they scatter to nothing -- no separate mask mult)
                    rank = cmp1.tile([P, TT], F32, name="rank", tag="rank")
                    nc.vector.tensor_tensor(rank, cum_t, M, ALU.subtract)
                    nc.vector.tensor_tensor(rank, rank, carry_ps, ALU.add)
                    nc.vector.tensor_scalar_add(rank, rank, 2048.0)
                    nc.vector.scalar_tensor_tensor(rank, M, -2048.0, rank,
                                                   ALU.mult, ALU.add)
                    rank_i = cmp1.tile([P, TT], I32, name="rank_i", tag="rank_i")
                    nc.vector.tensor_copy(rank_i, rank)
                    digi = cmp1.tile([P, 2, TT], I32, name="digi", tag="digi")
                    nc.vector.tensor_scalar(digi[:, 0, :], rank_i, 15, None,
                                            ALU.bitwise_and)
                    nc.vector.tensor_scalar(digi[:, 1, :], rank_i, 4, None,
                                            ALU.logical_shift_right)
                    dig = cmp1.tile([P, 2, TT], F16, name="dig", tag="dig")
                    nc.vector.tensor_copy(dig, digi)
                    digs.append(dig)
                    c16 = cmp1.tile([P, TT], F16, name="c16", tag="c16")
                    nc.vector.tensor_copy(c16, C2_sb[:, :, k])
                    c16s.append(c16)

                def compact_mms(k):
                    """Mask build + accumulating scatter matmuls + idx."""
                    dig = digs[k]
                    s16m = cmp.tile([P, TT, 16], F16, name=f"s16_{k}",
                                    tag="s16")
                    m48t = cmp.tile([P, TT, NC16], F16, name=f"m48a_{k}",
                                    tag="m48a")
                    m48c = cmp.tile([P, TT, NC16], F16, name=f"m48b_{k}",
                                    tag="m48b")
                    lo_b = dig[:, 0, :, None].broadcast_to([P, TT, 16])
                    hi_b = dig[:, 1, :, None].broadcast_to([P, TT, NC16])
                    C_b = c16s[k][:, :, None].broadcast_to([P, TT, NC16])
                    nc.vector.tensor_tensor(s16m, i16r, lo_b, ALU.is_equal)
                    # eq48 = (iota48 == rank//16); m48c = eq48*C (separate
                    # tile), then m48t *= tokid in place
                    nc.vector.tensor_tensor(m48t, i48r, hi_b, ALU.is_equal)
                    nc.vector.tensor_tensor(m48c, m48t, C_b, ALU.mult)
                    nc.vector.tensor_tensor(m48t, m48t, tokr, ALU.mult)
                    ids_t = zc.tile([P, NC16], F32, name="ids_t",
                                    tag="pz2")[0:16, :]
                    w_t = zc.tile([P, NC16], F32, name="w_t",
                                  tag="pz3")[0:16, :]
                    for tt in range(TT):
                        nc.tensor.matmul(ids_t, s16m[:, tt, :],
                                         m48t[:, tt, :],
                                         start=(tt == 0), stop=(tt == TT - 1))
                        nc.tensor.matmul(w_t, s16m[:, tt, :],
                                         m48c[:, tt, :],
                                         start=(tt == 0), stop=(tt == TT - 1))
                    ids_f = cmp1.tile([16, NC16], F32, name="ids_f", tag="ids_f")
                    nc.vector.tensor_copy(ids_f, ids_t)
                    nc.vector.tensor_copy(W16[k], w_t)
                    bc_ps = zc.tile([P, NC16], F32, name="bc_ps", tag="pz3")
                    nc.tensor.matmul(bc_ps, BI, ids_f, start=True, stop=True)
                    nc.vector.tensor_copy(idx16[k], bc_ps)
                    # gathers for this expert start as soon as idx is ready
                    halves = []
                    for hh, hcap in enumerate(_halves(CAPS[k])):
                        base = hh * ACH
                        xgh = ag.tile([P, KT_H, hcap], BF16, name=f"xg{k}_{hh}",
                                      tag="xg", bufs=2)
                        csl = slice(base // 16, (base + hcap) // 16)
                        nc.gpsimd.dma_gather(
                            xgh, xbf_d.ap(), idx16[k][:, csl],
                            hcap, hcap, H, transpose=True)
                        halves.append(xgh)
                    return halves

                def shared_c_group(hc, s, gi):
                    sw2q = sw2q_t[hc]
                    hsl = slice(hc * 512, (hc + 1) * 512)
                    ssl = slice(s * P, (s + 1) * P)
                    pz = zc.tile([P, 512], F32, name="spz", tag=f"pz{gi % 2}")
                    for ki in range(KT_I):
                        nc.tensor.matmul(pz, ys[:, ki, ssl], sw2q[:, ki, :],
                                         start=(ki == 0), stop=(ki == KT_I - 1))
                    ot = so.tile([P, 512], BF16, name="ot", tag="ot")
                    nc.vector.tensor_copy(ot, pz)
                    nc.scalar.dma_start(out=out_d.ap()[ssl, hsl], in_=ot)

                # PE fill order across the compaction window:
                xg = [None, None]
                xg[0] = compact_mms(0)
                xg[1] = compact_mms(1)
                shared_a_block(N_SA_ROUTER + 1, use_sigmoid=False)
                gi = 0
                for hc in range(4):
                    for s in range(TS // P):
                        shared_c_group(hc, s, gi)
                        gi += 1
                # exports on the scalar HWDGE queue: keeps the Q7 queue
                # free for the critical-path gathers
                for k in range(E_PER_CORE):
                    nc.sync.dma_start(out=ids_d.ap()[k], in_=idx16[k][0:16, :])
                    # W16[q, 8s + r] -> W128[r*16 + q, s]
                    for r in range(8):
                        nc.sync.dma_start(out=W128[k][16 * r:16 * (r + 1), :],
                                          in_=W16[k][:, r::8])

                # ---------------- routed FFN per expert ----------------
                y = [sres.tile([P, KT_I, capc[k]], BF16, name=f"y{k}")
                     for k in range(E_PER_CORE)]

                def pass_a_ch(k, c, h):
                    """pass A quarter: silu(x@w1T)*(x@w3T) for one (c, h)."""
                    hcap = [ACH, capc[k] - ACH][c]
                    w1h, w3h = w1h_t[(k, h)], w3h_t[(k, h)]
                    for m in range(4):
                        mi = h * 4 + m
                        msl = slice(m * P, (m + 1) * P)
                        pg = aps.tile([P, ACH], F32, name="pg",
                                      tag=f"pg{m % 2}")[:, :hcap]
                        pu = aps.tile([P, ACH], F32, name="pu",
                                      tag=f"pu{m % 2}")[:, :hcap]
                        for kt in range(KT_H):
                            nc.tensor.matmul(
                                pg, w1h[:, kt, msl], xg[k][c][:, kt, :hcap],
                                start=(kt == 0), stop=(kt == KT_H - 1))
                        for kt in range(KT_H):
                            nc.tensor.matmul(
                                pu, w3h[:, kt, msl], xg[k][c][:, kt, :hcap],
                                start=(kt == 0), stop=(kt == KT_H - 1))
                        sg = ay.tile([P, ACH], BF16, name="sg",
                                     tag="sg")[:, :hcap]
                        nc.scalar.activation(sg, pg, AF.Silu)
                        csl = slice(c * ACH, c * ACH + hcap)
                        nc.vector.tensor_tensor(y[k][:, mi, csl], sg,
                                                pu, ALU.mult)

                def pass_c_q(k, q, gi0):
                    """pass C for one q block: z = W * (y @ w2T)."""
                    gi = gi0
                    ns_full, rem = divmod(capc[k], P)
                    sdims = [P] * ns_full + ([rem] if rem else [])
                    w2q = w2h_t[(k, q)]
                    hsl = slice(q * 512, (q + 1) * 512)
                    for s, sdim in enumerate(sdims):
                        ssl = slice(s * P, s * P + sdim)
                        pz = zc.tile([P, 512], F32, name="pz",
                                     tag=f"pz{gi % 4}")[:sdim, :]
                        gi += 1
                        for ki in range(KT_I):
                            nc.tensor.matmul(pz, y[k][:, ki, ssl],
                                             w2q[:, ki, :],
                                             start=(ki == 0),
                                             stop=(ki == KT_I - 1))
                        zt = zo.tile([P, 512], BF16, name="zc",
                                     tag="zc")[:sdim, :]
                        nc.vector.tensor_scalar_mul(zt, pz,
                                                    W128[k][:sdim, s:s + 1])
                        nc.sync.dma_start(out=z_d.ap()[k, ssl, hsl], in_=zt)
                    return gi

                # pass A e0, then pass C e0 interleaved with pass A e1 so
                # the per-engine FIFOs (PE, vector, scalar) overlap the
                # two experts instead of head-of-line blocking
                for c in range(2):
                    for h in range(2):
                        pass_a_ch(0, c, h)
                gi = pass_c_q(0, 0, 0)
                pass_a_ch(1, 0, 0)
                gi = pass_c_q(0, 1, gi)
                pass_a_ch(1, 0, 1)
                gi = pass_c_q(0, 2, gi)
                pass_a_ch(1, 1, 0)
                gi = pass_c_q(0, 3, gi)
                pass_a_ch(1, 1, 1)
                gi = 0
                for q in range(4):
                    gi = pass_c_q(1, q, gi)

            es_.close()

    nc.compile()
    return nc


_NC_CACHE = {}


def _get_nc(capc):
    if capc not in _NC_CACHE:
        _NC_CACHE[capc] = build_nc(capc)
    return _NC_CACHE[capc]


def _route_counts(x, gate_w, expert_bias):
    """Host-side routing counts, used ONLY for load-balanced expert->core
    assignment and compute-capacity sizing (sharding decisions); the
    device recomputes routing."""
    logits = x @ gate_w.T
    scores = 1.0 / (1.0 + np.exp(-logits))
    sel = scores + expert_bias[None, :]
    grp = sel.reshape(T, G, E // G)
    t2 = np.sort(grp, -1)[:, :, -2:].sum(-1)
    gidx = np.argsort(t2, -1)[:, -2:]
    gmask = np.zeros((T, G), bool)
    gmask[np.arange(T)[:, None], gidx] = True
    emask = np.repeat(gmask, E // G, axis=1)
    masked = np.where(emask, sel, -np.inf)
    ids = np.argsort(masked, -1)[:, -K_TOP:]
    return np.bincount(ids.ravel(), minlength=E)


def _pc(aT, ncol):
    """[KT*128, C] -> [C//ncol, 128, KT*ncol] partition-contiguous."""
    kt = aT.shape[0] // P
    nch = aT.shape[1] // ncol
    a = aT.reshape(kt, P, nch, ncol)
    return np.ascontiguousarray(
        np.transpose(a, (2, 1, 0, 3)).reshape(nch, P, kt * ncol))


def kernel(hidden_states, gate_w, expert_bias, w1, w3, w2, sw1, sw3, sw2):
    x = np.ascontiguousarray(hidden_states, dtype=np.float32)
    bf = ml_dtypes.bfloat16
    xhi = x.astype(bf)
    xlo = (x - xhi.astype(np.float32)).astype(bf)
    gw = np.ascontiguousarray(gate_w.astype(np.float32))
    ghi = gw.astype(bf)
    glo = (gw - ghi.astype(np.float32)).astype(bf)
    xbf = np.ascontiguousarray(xhi)
    xhiT = np.ascontiguousarray(xhi.T)
    xloT = np.ascontiguousarray(xlo.T)
    bias = expert_bias.astype(np.float32)
    biasb = np.ascontiguousarray(np.broadcast_to(bias[None, :], (P, E)))

    # partition-contiguous streaming layouts
    xhi_r = _pc(xhiT, TCH)                      # [NTOK, P, KT_H*TCH]
    xlo_r = _pc(xloT, TCH)
    gpair = np.concatenate([np.ascontiguousarray(ghi.T),
                            np.zeros((H, E), ghi.dtype),
                            np.ascontiguousarray(glo.T)], axis=1)
    g_r = _pc(np.ascontiguousarray(gpair), 3 * E)[0]
    sw13_r = np.stack([_pc(np.ascontiguousarray(sw1.T.astype(bf)), 256),
                       _pc(np.ascontiguousarray(sw3.T.astype(bf)), 256)])
    sw2_r = _pc(np.ascontiguousarray(sw2.T.astype(bf)), 512)

    # load-balanced assignment: pair i-th largest with i-th smallest
    counts = _route_counts(x.astype(np.float64), gw.astype(np.float64),
                           bias.astype(np.float64))
    order = np.argsort(-counts)
    assign = [(int(order[i]), int(order[E - 1 - i])) for i in range(N_CORES)]
    # compute capacity: actual max per slot + margin, rounded to 8
    cnt0 = max(counts[a] for a, _ in assign)
    cnt1 = max(counts[b] for _, b in assign)
    capc = (min(CAPS[0], max(ACH + 8, -(-(int(cnt0) + 8) // 8) * 8)),
            min(CAPS[1], max(ACH + 8, -(-(int(cnt1) + 8) // 8) * 8)))

    w1tb = np.transpose(w1, (0, 2, 1)).astype(bf)
    w3tb = np.transpose(w3, (0, 2, 1)).astype(bf)
    w2tb = np.transpose(w2, (0, 2, 1)).astype(bf)

    in_maps = []
    for c in range(N_CORES):
        e_hi, e_lo = assign[c]
        esel = np.zeros((P, 2, E), np.float32)
        esel[:, 0, e_hi] = 1.0
        esel[:, 1, e_lo] = 1.0
        pick = [e_hi, e_lo]
        w13_r = np.stack([
            np.stack([
                np.stack([_pc(np.ascontiguousarray(w1tb[e]), 512),
                          _pc(np.ascontiguousarray(w3tb[e]), 512)], axis=1)[h]
                for h in range(2)])
            for e in pick])                     # [2, 2, 2, P, KT_H*512]
        w2_r = np.stack([_pc(np.ascontiguousarray(w2tb[e]), 512)
                         for e in pick])        # [2, 4, P, KT_I*512]
        xbs_r = _pc(np.ascontiguousarray(xhiT[:, TS * c:TS * (c + 1)]), TS)[0]
        in_maps.append({
            "xhi": xhi_r,
            "xlo": xlo_r,
            "xbf": xbf,
            "g": g_r,
            "biasb": biasb,
            "esel": esel,
            "w13": np.ascontiguousarray(w13_r),
            "w2": np.ascontiguousarray(w2_r),
            "sw13": sw13_r,
            "sw2": sw2_r,
            "xbs": xbs_r,
        })

    nc = _get_nc(capc)
    res = run_bass_kernel_spmd(nc, in_maps, list(range(N_CORES)))

    out = np.zeros((T, H), np.float32)
    for c in range(N_CORES):
        r = res.results[c]
        z = np.asarray(r["z"], dtype=np.float32)          # [2, CAPS[0], H]
        ids = np.asarray(r["ids"], dtype=np.int64)        # [2, 16, NC16]
        for k in range(E_PER_CORE):
            slot_ids = ids[k].T.reshape(-1)               # slot i at [i%16, i//16]
            nz = np.nonzero(slot_ids)[0]
            cnt = (nz[-1] + 1) if len(nz) else 0
            cnt = min(cnt, capc[k])
            if cnt:
                out[slot_ids[:cnt]] += z[k, :cnt]
        out[TS * c:TS * (c + 1)] += np.asarray(r["out"], dtype=np.float32)
    kernel.last_result = res
    return out


# revision 47
# speedup vs baseline: 1.1383x; 1.0188x over previous
"""MoE (BailingMoeV2.5) Trainium2 kernel — 8-core expert-parallel, SPARSE.

T=2048 tokens, H=2048 hidden, E=16 experts (4 groups, top-2 groups,
top-4 experts), I=1024 expert intermediate, shared expert IS=1024,
routed scale 2.5.

Each core owns 2 experts (host pairs high-count with low-count experts;
gather slot capacities 768/640, compute capacities trimmed to the
actual max routed counts + margin, rounded to 8):
  1. Router: logits via lossless-ish bf16 hi/lo split (3 bf16 passes),
     sigmoid scores, batched grouped top-k epilogue (3 pieces: 8/4/4
     token-tiles, overlapped with the score stream) -> per-token
     combine weights C2[token, 2] for this core's experts (x2.5,
     renormalized).
  2. Device-side stream compaction per expert (cumsum-matmul rank +
     batched one-hot mask build via big tensor_tensor ops + fused
     onehot matmuls) -> token-id list (int16, dma_gather layout) +
     per-slot weights.  Padding slots gather token 0 with W=0.
  3. dma_gather (transpose mode) pulls selected tokens from the bf16
     token-major x into feature-major [128, 16, cap].
  4. bf16 SwiGLU FFN per expert over the COMPUTE capacity only;
     output scaled by W -> z + ids exported.
  5. Shared expert (bf16) on the core's 256-token slice; shared-A
     blocks 0-4 interleave with router chunks (silu computed as
     x*sigmoid(x) to avoid ACT-table thrash), blocks 5-7 + shared
     pass C fill the compaction window.
Host unshard: out[ids] += z per (core, slot); out[slice_c] += shared_c.

DMA queues: sync(SP) = xhi router chunks then routed weights;
scalar(Act) = chunk-0 split + shared-expert feeds then outputs;
gpsimd = xlo router chunks, idx bookkeeping + gathers.
All large streams use host-side partition-contiguous layouts
(one >=4KB contiguous run per partition per DMA).
"""
import sys
from contextlib import ExitStack

sys.path.insert(0, "/opt/trn_rl_repo")

import numpy as np
import ml_dtypes

import concourse.bass as bass
import concourse.mybir as mybir
import concourse.tile as tile
from concourse import bacc
from concourse.bass_utils import run_bass_kernel_spmd
from concourse.masks import make_identity, make_upper_triangular

P = 128
T, H, E, K_TOP, I = 2048, 2048, 16, 4, 1024
G = 4
IS = 1024
N_CORES = 8
E_PER_CORE = E // N_CORES  # 2
TS = T // N_CORES          # 256
ROUTED_SCALE = 2.5

KT_H = H // P              # 16
KT_I = I // P              # 8
NTOK = 8                   # router token chunks of 256
TCH = T // NTOK            # 256
TT = T // P                # 16
CAPS = (768, 640)          # gather slot capacity (multiple of 128)
NC16 = 48                  # idx cols allocated (CAPS[0]/16)
ACH = 384                  # pass-A slot chunk (psum bank fits 384 fp32)
N_SA_ROUTER = 6            # shared-A blocks interleaved with router

F32 = mybir.dt.float32
F16 = mybir.dt.float16
BF16 = mybir.dt.bfloat16
I16 = mybir.dt.int16
I32 = mybir.dt.int32
AX = mybir.AxisListType.X
ALU = mybir.AluOpType
AF = mybir.ActivationFunctionType


def _halves(cap):
    return (ACH, cap - ACH)


def build_nc(capc):
    """capc: per-slot COMPUTE capacity (<= CAPS, multiple of 8)."""
    nc = bacc.Bacc(None, target_bir_lowering=False, debug=False)

    # all streaming inputs are partition-contiguous: [.., P, inner]
    xhi_d = nc.declare_dram_parameter("xhi", [NTOK, P, KT_H * TCH], BF16, isOutput=False)
    xlo_d = nc.declare_dram_parameter("xlo", [NTOK, P, KT_H * TCH], BF16, isOutput=False)
    xbf_d = nc.declare_dram_parameter("xbf", [T, H], BF16, isOutput=False)
    g_d = nc.declare_dram_parameter("g", [P, KT_H * 3 * E], BF16, isOutput=False)
    biasb_d = nc.declare_dram_parameter("biasb", [P, E], F32, isOutput=False)
    esel_d = nc.declare_dram_parameter("esel", [P, 2, E], F32, isOutput=False)
    w13_d = nc.declare_dram_parameter("w13", [E_PER_CORE, 2, 2, P, KT_H * 512], BF16, isOutput=False)
    w2_d = nc.declare_dram_parameter("w2", [E_PER_CORE, 4, P, KT_I * 512], BF16, isOutput=False)
    sw13_d = nc.declare_dram_parameter("sw13", [2, 4, P, KT_H * 256], BF16, isOutput=False)
    sw2_d = nc.declare_dram_parameter("sw2", [4, P, KT_I * 512], BF16, isOutput=False)
    xbs_d = nc.declare_dram_parameter("xbs", [P, KT_H * TS], BF16, isOutput=False)

    z_d = nc.declare_dram_parameter("z", [E_PER_CORE, CAPS[0], H], BF16, isOutput=True)
    ids_d = nc.declare_dram_parameter("ids", [E_PER_CORE, 16, NC16], I16, isOutput=True)
    out_d = nc.declare_dram_parameter("out", [TS, H], BF16, isOutput=True)

    def _c3(ap, n):
        return ap.rearrange("p (kt n) -> p kt n", n=n)

    with tile.TileContext(nc) as tc:
        with tc.tile_pool(name="res", bufs=1) as res:
            # ---------------- persistent small tiles ----------------
            sc_all = res.tile([P, TT, E], F32, name="sc_all")
            C2_sb = res.tile([P, TT, E_PER_CORE], F32, name="C2_sb")
            M2_sb = res.tile([P, TT, E_PER_CORE], F32, name="M2_sb")
            ident = res.tile([P, P], F32, name="ident")
            make_identity(nc, ident)
            tril = res.tile([P, P], F32, name="tril")
            make_upper_triangular(nc, tril, val=1.0, diag=True)
            ones128p = res.tile([P, 1], F32, name="ones128p")
            nc.vector.memset(ones128p, 1.0)
            ones_row = res.tile([1, P], F32, name="ones_row")
            nc.vector.memset(ones_row, 1.0)
            iotas = res.tile([P, 80], F32, name="iotas")
            iota16 = iotas[:, 0:16]
            iota48 = iotas[:, 16:64]
            tokid = iotas[:, 64:80]
            ii = res.tile([P, NC16], I32, name="ii")
            nc.gpsimd.iota(ii[:, 0:16], pattern=[[1, 16]], base=0, channel_multiplier=0)
            nc.vector.tensor_copy(iota16, ii[:, 0:16])
            nc.gpsimd.iota(ii[:, 0:NC16], pattern=[[1, NC16]], base=0, channel_multiplier=0)
            nc.vector.tensor_copy(iota48, ii[:, 0:NC16])
            nc.gpsimd.iota(ii[:, 0:TT], pattern=[[P, TT]], base=0, channel_multiplier=1)
            nc.vector.tensor_copy(tokid, ii[:, 0:TT])

            idx16 = [res.tile([P, NC16], I16, name=f"idx16_{k}")
                     for k in range(E_PER_CORE)]
            W128 = [res.tile([P, 6], F32, name=f"W128_{k}")
                    for k in range(E_PER_CORE)]
            W16 = [res.tile([16, NC16], F32, name=f"W16_{k}")
                   for k in range(E_PER_CORE)]
            # iota/token-id repeats for the batched compaction masks,
            # built up-front while the engines are otherwise idle
            i16r = res.tile([P, TT, 16], F16, name="i16r")
            nc.vector.tensor_copy(
                i16r, iota16[:, None, :].broadcast_to([P, TT, 16]))
            i48r = res.tile([P, TT, NC16], F16, name="i48r")
            nc.vector.tensor_copy(
                i48r, iota48[:, None, :].broadcast_to([P, TT, NC16]))
            tokr = res.tile([P, TT, NC16], F16, name="tokr")
            nc.vector.tensor_copy(
                tokr, tokid[:, :, None].broadcast_to([P, TT, NC16]))
            # block-identity BI[q, p] = (p % 16 == q), for idx broadcast
            BI = res.tile([16, P], F32, name="BI")
            bii = res.tile([16, P], I32, name="bii")
            nc.gpsimd.iota(bii, pattern=[[1, P]], base=0, channel_multiplier=0)
            nc.vector.tensor_scalar(bii, bii, 15, None, ALU.bitwise_and)
            bif = res.tile([16, P], F32, name="bif")
            nc.vector.tensor_copy(bif, bii)
            qcolf = res.tile([16, 1], F32, name="qcolf")
            qcol = res.tile([16, 1], I32, name="qcol")
            nc.gpsimd.iota(qcol, pattern=[[1, 1]], base=0, channel_multiplier=1)
            nc.vector.tensor_copy(qcolf, qcol)
            nc.vector.tensor_scalar(BI, bif, qcolf, None, ALU.is_equal)

            # shared-expert pools at outer scope
            es_ = ExitStack()
            swp = es_.enter_context(tc.tile_pool(name="sw", bufs=3))
            sres = es_.enter_context(tc.tile_pool(name="sres", bufs=1))
            so = es_.enter_context(tc.tile_pool(name="so", bufs=2))
            aps = es_.enter_context(tc.tile_pool(name="aps", bufs=1, space="PSUM"))
            # scalar (Act) HWDGE queue: chunk-0 xhi halves first (the
            # sync queue starts with g weights + chunks 1-7), then the
            # shared-expert feeds.
            es0 = ExitStack()
            rx0 = es0.enter_context(tc.tile_pool(name="rx0", bufs=1))
            xh0a = rx0.tile([P, KT_H // 2, TCH], BF16, name="xh0a")
            xh0b = rx0.tile([P, KT_H // 2, TCH], BF16, name="xh0b")
            xl0a = rx0.tile([P, KT_H // 2, TCH], BF16, name="xl0a")
            xl0b = rx0.tile([P, KT_H // 2, TCH], BF16, name="xl0b")
            nc.scalar.dma_start(out=xh0a, in_=_c3(xhi_d.ap()[0], TCH)[:, 0:KT_H // 2, :])
            nc.gpsimd.dma_start(out=xl0a, in_=_c3(xlo_d.ap()[0], TCH)[:, 0:KT_H // 2, :])
            nc.scalar.dma_start(out=xl0b, in_=_c3(xlo_d.ap()[0], TCH)[:, KT_H // 2:, :])
            # shared expert feeds on scalar queue
            xs = sres.tile([P, KT_H, TS], BF16, name="xs")
            nc.scalar.dma_start(out=xs, in_=_c3(xbs_d.ap(), TS))
            # sw13 loads self-throttle via the swx slot rotation (later
            # tiles wait on earlier shared-A blocks); sw2 queues behind
            # them so its 4MB stays out of the router-chunk congestion
            # window but lands before shared-C needs it.
            sw1q_t, sw3q_t, sw2q_t = {}, {}, {}
            for q in range(4):
                sw1q_t[q] = swp.tile([P, KT_H, 256], BF16, name="sw1q",
                                     tag="swx", bufs=2)
                sw3q_t[q] = swp.tile([P, KT_H, 256], BF16, name="sw3q",
                                     tag="swx", bufs=2)
                nc.scalar.dma_start(out=sw1q_t[q], in_=_c3(sw13_d.ap()[0, q], 256))
                nc.scalar.dma_start(out=sw3q_t[q], in_=_c3(sw13_d.ap()[1, q], 256))
            for q in range(4):
                sw2q_t[q] = swp.tile([P, KT_I, 512], BF16, name="sw2q",
                                     tag="sw2", bufs=3)
                nc.scalar.dma_start(out=sw2q_t[q], in_=_c3(sw2_d.ap()[q], 512))
            ys = sres.tile([P, KT_I, TS], BF16, name="ys")

            def shared_a_block(mi, use_sigmoid):
                h, m = mi // 2, mi % 2
                sw1h, sw3h = sw1q_t[h], sw3q_t[h]
                msl = slice(m * P, (m + 1) * P)
                pg = aps.tile([P, ACH], F32, name="spg",
                              tag=f"pg{mi % 2}")[:, :TS]
                pu = aps.tile([P, ACH], F32, name="spu",
                              tag=f"pu{mi % 2}")[:, :TS]
                for kt in range(KT_H):
                    nc.tensor.matmul(pg, sw1h[:, kt, msl], xs[:, kt, :],
                                     start=(kt == 0), stop=(kt == KT_H - 1))
                for kt in range(KT_H):
                    nc.tensor.matmul(pu, sw3h[:, kt, msl], xs[:, kt, :],
                                     start=(kt == 0), stop=(kt == KT_H - 1))
                sg = so.tile([P, TS], BF16, name="ssg", tag="ssg")
                if use_sigmoid:
                    # silu(x) = x * sigmoid(x): avoids Sigmoid<->Silu
                    # ACT-table reloads between router chunks
                    nc.scalar.activation(sg, pg, AF.Sigmoid)
                    st = so.tile([P, TS], BF16, name="sst", tag="sst")
                    nc.vector.tensor_tensor(st, sg, pu, ALU.mult)
                    nc.vector.tensor_tensor(ys[:, mi, :], st, pg, ALU.mult)
                else:
                    nc.scalar.activation(sg, pg, AF.Silu)
                    nc.vector.tensor_tensor(ys[:, mi, :], sg, pu, ALU.mult)

            # =================== router (bf16 hi/lo) ===================
            with tc.tile_pool(name="rt", bufs=2) as rt, \
                 tc.tile_pool(name="rt1", bufs=1) as rt1, \
                 tc.tile_pool(name="rxn", bufs=2) as rxn, \
                 tc.tile_pool(name="rtp", bufs=2, space="PSUM") as rtp:
                # gcat[:, kt, 0:16] = ghi, [:, kt, 32:48] = glo (16:32
                # zero pad): one M=48 stationary pass computes ghi@xh and
                # glo@xh together; the pad keeps glo's psum rows at base
                # partition 32 (engine partition-offset constraint)
                gcat = rt1.tile([P, KT_H, 3 * E], BF16, name="gcat")
                nc.sync.dma_start(out=gcat, in_=_c3(g_d.ap(), 3 * E))
                nc.sync.dma_start(out=xh0b,
                                  in_=_c3(xhi_d.ap()[0], TCH)[:, KT_H // 2:, :])
                biasb = rt1.tile([P, E], F32, name="biasb")
                nc.sync.dma_start(out=biasb, in_=biasb_d.ap())
                esel = rt1.tile([P, 2, E], F32, name="esel")
                nc.sync.dma_start(out=esel, in_=esel_d.ap())
                sT = rt1.tile([16, T], F32, name="sT")

                def epilogue_part(ts0, nts):
                    """Grouped top-k for tt in [ts0, ts0+nts) -> C2/M2."""
                    tsl = slice(ts0, ts0 + nts)
                    sc = sc_all[:, tsl, :]
                    selA = rt.tile([P, 8, E], F32, name="selA",
                                   tag="selA")[:, :nts, :]
                    nc.vector.tensor_tensor(
                        selA, sc,
                        biasb[:, None, :].broadcast_to([P, nts, E]), ALU.add)
                    a = selA[:, :, 0::4]
                    b = selA[:, :, 1::4]
                    c_ = selA[:, :, 2::4]
                    d = selA[:, :, 3::4]
                    t4 = rt.tile([P, 8, 6, G], F32, name="t4",
                                 tag="t4")[:, :nts, :, :]
                    m1, n1, m2, n2, gs, tmp = (t4[:, :, j, :] for j in range(6))
                    nc.vector.tensor_tensor(m1, a, b, ALU.max)
                    nc.vector.tensor_tensor(n1, a, b, ALU.min)
                    nc.vector.tensor_tensor(m2, c_, d, ALU.max)
                    nc.vector.tensor_tensor(n2, c_, d, ALU.min)
                    nc.vector.tensor_tensor(gs, m1, m2, ALU.add)
                    nc.vector.tensor_tensor(tmp, m1, n1, ALU.add)
                    nc.vector.tensor_tensor(gs, gs, tmp, ALU.max)
                    nc.vector.tensor_tensor(tmp, m2, n2, ALU.add)
                    nc.vector.tensor_tensor(gs, gs, tmp, ALU.max)
                    g2 = rt.tile([P, 8, 6], F32, name="g2",
                                 tag="g2")[:, :nts, :]
                    ga, gb = gs[:, :, 0::2], gs[:, :, 1::2]
                    gmx, gmn = g2[:, :, 0:2], g2[:, :, 2:4]
                    gthr = g2[:, :, 4:5]
                    gt2 = g2[:, :, 5:6]
                    nc.vector.tensor_tensor(gmx, ga, gb, ALU.max)
                    nc.vector.tensor_tensor(gmn, ga, gb, ALU.min)
                    nc.vector.tensor_tensor(gthr, gmx[:, :, 0:1], gmx[:, :, 1:2],
                                            ALU.min)
                    nc.vector.tensor_tensor(gt2, gmn[:, :, 0:1], gmn[:, :, 1:2],
                                            ALU.max)
                    nc.vector.tensor_tensor(gthr, gthr, gt2, ALU.max)
                    gmask = rt.tile([P, 8, G], F32, name="gmask",
                                    tag="gmask")[:, :nts, :]
                    nc.vector.tensor_tensor(
                        gmask, gs, gthr.broadcast_to([P, nts, G]), ALU.is_ge)
                    emask = rt.tile([P, 8, E], F32, name="emask",
                                    tag="emask")[:, :nts, :]
                    for j in range(4):
                        nc.vector.tensor_copy(emask[:, :, j::4], gmask)
                    masked = rt.tile([P, 8, E], F32, name="masked",
                                     tag="masked")[:, :nts, :]
                    nc.vector.tensor_scalar_add(emask, emask, -1.0)
                    nc.vector.scalar_tensor_tensor(masked, emask, 1e30, selA,
                                                   ALU.mult, ALU.add)
                    m8s = rt.tile([P, 8, 8], F32, name="m8s",
                                  tag="m8s")[:, :nts, :]
                    for tt in range(nts):
                        nc.vector.max(m8s[:, tt, :], masked[:, tt, :])
                    selm = rt.tile([P, 8, E], F32, name="selm",
                                   tag="selm")[:, :nts, :]
                    nc.vector.tensor_tensor(
                        selm, masked,
                        m8s[:, :, 3:4].broadcast_to([P, nts, E]), ALU.is_ge)
                    cw = rt.tile([P, 8, E], F32, name="cw",
                                 tag="cw")[:, :nts, :]
                    nc.vector.tensor_tensor(cw, sc, selm, ALU.mult)
                    den = rt.tile([P, 8, 2], F32, name="den",
                                  tag="den")[:, :nts, :]
                    nc.vector.reduce_sum(den[:, :, 0:1], cw, AX)
                    nc.vector.tensor_scalar_add(den[:, :, 0:1], den[:, :, 0:1],
                                                1e-20)
                    nc.vector.reciprocal(den[:, :, 1:2], den[:, :, 0:1])
                    nc.vector.tensor_scalar_mul(den[:, :, 1:2], den[:, :, 1:2],
                                                ROUTED_SCALE)
                    nc.vector.tensor_tensor(
                        cw, cw, den[:, :, 1:2].broadcast_to([P, nts, E]), ALU.mult)
                    esm = rt.tile([P, 8, E], F32, name="esm",
                                  tag="esm")[:, :nts, :]
                    for k in range(E_PER_CORE):
                        nc.vector.tensor_tensor(
                            esm, cw,
                            esel[:, k, :][:, None, :].broadcast_to([P, nts, E]),
                            ALU.mult)
                        nc.vector.reduce_sum(C2_sb[:, tsl, k:k + 1], esm, AX)
                    nc.vector.tensor_scalar(
                        M2_sb[:, tsl, :].rearrange("p a b -> p (a b)"),
                        C2_sb[:, tsl, :].rearrange("p a b -> p (a b)"),
                        0.0, None, ALU.is_gt)

                for n in range(NTOK):
                    if n == 0:
                        xh_parts = [(xh0a, 0), (xh0b, KT_H // 2)]
                        xl_parts = [(xl0a, 0), (xl0b, KT_H // 2)]
                    else:
                        xh = rxn.tile([P, KT_H, TCH], BF16, name="xh",
                                      tag="xh", bufs=3)
                        xl = rxn.tile([P, KT_H, TCH], BF16, name="xl",
                                      tag="xl", bufs=1)
                        nc.sync.dma_start(out=xh, in_=_c3(xhi_d.ap()[n], TCH))
                        nc.gpsimd.dma_start(out=xl, in_=_c3(xlo_d.ap()[n], TCH))
                        xh_parts = [(xh, 0)]
                        xl_parts = [(xl, 0)]
                    # emit the shared-A filler BEFORE this chunk's MMs:
                    # if the chunk's data is late, the in-order PE queue
                    # can still run the (independent) filler block
                    if 1 <= n <= N_SA_ROUTER + 1:
                        shared_a_block(n - 1, use_sigmoid=True)
                    tksl = slice(n * TCH, (n + 1) * TCH)
                    ps = rtp.tile([48, TCH], F32, name="ps_r", tag="ps_r")
                    # pass 1: [ghi|glo] @ xh -> rows 0:32; pass 2:
                    # ghi @ xl accumulates into rows 0:16
                    for pi, (x_, koff) in enumerate(xh_parts):
                        nkt = x_.shape[1]
                        for kt in range(nkt):
                            nc.tensor.matmul(
                                ps, gcat[:, koff + kt, :], x_[:, kt, :],
                                start=(pi == 0 and kt == 0), stop=False)
                    nl = len(xl_parts)
                    for pi, (x_, koff) in enumerate(xl_parts):
                        nkt = x_.shape[1]
                        for kt in range(nkt):
                            nc.tensor.matmul(
                                ps[0:16, :], gcat[:, koff + kt, 0:E],
                                x_[:, kt, :],
                                start=False,
                                stop=(pi == nl - 1 and kt == nkt - 1))
                    s2 = rt.tile([16, 2, TCH], F32, name="s2", tag="s2")
                    nc.vector.tensor_copy(s2[:, 1, :], ps[32:48, :])
                    nc.vector.tensor_tensor(s2[:, 0, :], ps[0:16, :],
                                            s2[:, 1, :], ALU.add)
                    nc.scalar.activation(sT[:, tksl], s2[:, 0, :], AF.Sigmoid)
                    for tt in range(2 * n, 2 * n + 2):
                        pst = rtp.tile([P, 16], F32, name="pst", tag="pst")
                        nc.tensor.transpose(pst, sT[:, tt * P:(tt + 1) * P],
                                            ident[:16, :16])
                        nc.vector.tensor_copy(sc_all[:, tt, :], pst)
                    if n == 3:
                        epilogue_part(0, 8)
                    elif n == 5:
                        epilogue_part(8, 4)
                    elif n == 6:
                        epilogue_part(12, 2)
                epilogue_part(14, 2)
            es0.close()   # free chunk-0 tiles before FFN pools allocate

            # ============ compaction + shared + routed FFN ============
            # PSUM banks (8): aps 4 (pg0,pg1,pu0,pu1; shared-A + routed A),
            # zc 4 (pz0..pz3): shared-C on pz0/pz1, compaction accum on
            # pz2/pz3, routed C cycles all four.
            with tc.tile_pool(name="cmp", bufs=1) as cmp, \
                 tc.tile_pool(name="cmp1", bufs=2) as cmp1, \
                 tc.tile_pool(name="zc", bufs=1, space="PSUM") as zc, \
                 tc.tile_pool(name="aw", bufs=4) as aw, \
                 tc.tile_pool(name="w2p", bufs=2) as w2p, \
                 tc.tile_pool(name="ay", bufs=2) as ay, \
                 tc.tile_pool(name="ag", bufs=2) as ag, \
                 tc.tile_pool(name="zo", bufs=2) as zo:

                # sync (SP) HWDGE queue (behind router xhi stream):
                # routed weights, ordered by first need
                w1h_t, w3h_t, w2h_t = {}, {}, {}

                def _w13(k, h):
                    w1h = aw.tile([P, KT_H, 512], BF16, name="w1h", tag="wA")
                    w3h = aw.tile([P, KT_H, 512], BF16, name="w3h", tag="wA")
                    nc.sync.dma_start(out=w1h, in_=_c3(w13_d.ap()[k, h, 0], 512))
                    nc.sync.dma_start(out=w3h, in_=_c3(w13_d.ap()[k, h, 1], 512))
                    w1h_t[(k, h)] = w1h
                    w3h_t[(k, h)] = w3h

                def _w2(k, q):
                    w2q = w2p.tile([P, KT_I, 512], BF16, name="w2q", tag="w2")
                    nc.sync.dma_start(out=w2q, in_=_c3(w2_d.ap()[k, q], 512))
                    w2h_t[(k, q)] = w2q

                _w13(0, 0)
                _w13(0, 1)
                _w2(0, 0)
                _w2(0, 1)
                _w13(1, 0)
                _w13(1, 1)
                _w2(0, 2)
                _w2(0, 3)
                for q in range(4):
                    _w2(1, q)

                # ---- compaction: rank chains + batched mask build ----
                # phase a (both experts): rank via cumsum matmuls + scan
                # + digit split; then per expert: one-hot masks for ALL
                # 16 token tiles in a few large vector ops (fp16),
                # scatter matmuls, idx broadcast, gathers.  shared-A
                # block 6 leads so the PE has work while the vector
                # engine runs the final epilogue piece.
                shared_a_block(N_SA_ROUTER + 1, use_sigmoid=False)
                digs, c16s = [], []
                for k in range(E_PER_CORE):
                    M = M2_sb[:, :, k]
                    cum_t = zc.tile([P, NC16], F32, name="cum_t",
                                    tag="pz2")[:, 0:TT]
                    cmt = zc.tile([P, NC16], F32, name="cmt", tag="pz3")
                    tot_ps = cmt[0:1, 0:TT]
                    nc.tensor.matmul(cum_t, tril, M, start=True, stop=True)
                    nc.tensor.matmul(tot_ps, ones128p, M, start=True, stop=True)
                    tot = cmp1.tile([1, 3, TT], F32, name="tot", tag="tot")
                    ex0, ex1 = tot[:, 1, :], tot[:, 2, :]
                    nc.vector.tensor_copy(tot[:, 0, :], tot_ps)
                    nc.vector.memset(ex0[:, 0:1], 0.0)
                    nc.vector.tensor_copy(ex0[:, 1:], tot[:, 0, 0:TT - 1])
                    nc.vector.tensor_tensor_scan(ex1, ex0, ex0, 0.0,
                                                 ALU.add, ALU.bypass)
                    carry_ps = cmt[:, TT:2 * TT]
                    nc.tensor.matmul(carry_ps, ones_row, ex1, start=True, stop=True)
                    # rank, with non-routed tokens pushed out of range
                    # (+2048: keeps rank%16, sends rank//16 beyond 47, so
                    # they scatter to nothing -- no separate mask mult)
                    rank = cmp1.tile([P, TT], F32, name="rank", tag="rank")
                    nc.vector.tensor_tensor(rank, cum_t, M, ALU.subtract)
                    nc.vector.tensor_tensor(rank, rank, carry_ps, ALU.add)
                    nc.vector.tensor_scalar_add(rank, rank, 2048.0)
                    nc.vector.scalar_tensor_tensor(rank, M, -2048.0, rank,
                                                   ALU.mult, ALU.add)
                    rank_i = cmp1.tile([P, TT], I32, name="rank_i", tag="rank_i")
                    nc.vector.tensor_copy(rank_i, rank)
                    digi = cmp1.tile([P, 2, TT], I32, name="digi", tag="digi")
                    nc.vector.tensor_scalar(digi[:, 0, :], rank_i, 15, None,
                                            ALU.bitwise_and)
                    nc.vector.tensor_scalar(digi[:, 1, :], rank_i, 4, None,
                                            ALU.logical_shift_right)
                    dig = cmp1.tile([P, 2, TT], F16, name="dig", tag="dig")
                    nc.vector.tensor_copy(dig, digi)
                    digs.append(dig)
                    c16 = cmp1.tile([P, TT], F16, name="c16", tag="c16")
                    nc.vector.tensor_copy(c16, C2_sb[:, :, k])
                    c16s.append(c16)

                def compact_mms(k):
                    """Mask build + accumulating scatter matmuls + idx."""
                    dig = digs[k]
                    s16m = cmp.tile([P, TT, 16], F16, name=f"s16_{k}",
                                    tag="s16")
                    m48t = cmp.tile([P, TT, NC16], F16, name=f"m48a_{k}",
                                    tag="m48a")
                    m48c = cmp.tile([P, TT, NC16], F16, name=f"m48b_{k}",
                                    tag="m48b")
                    lo_b = dig[:, 0, :, None].broadcast_to([P, TT, 16])
                    hi_b = dig[:, 1, :, None].broadcast_to([P, TT, NC16])
                    C_b = c16s[k][:, :, None].broadcast_to([P, TT, NC16])
                    nc.vector.tensor_tensor(s16m, i16r, lo_b, ALU.is_equal)
                    # eq48 = (iota48 == rank//16); m48c = eq48*C (separate
                    # tile), then m48t *= tokid in place
                    nc.vector.tensor_tensor(m48t, i48r, hi_b, ALU.is_equal)
                    nc.vector.tensor_tensor(m48c, m48t, C_b, ALU.mult)
                    nc.vector.tensor_tensor(m48t, m48t, tokr, ALU.mult)
                    ids_t = zc.tile([P, NC16], F32, name="ids_t",
                                    tag="pz2")[0:16, :]
                    w_t = zc.tile([P, NC16], F32, name="w_t",
                                  tag="pz3")[0:16, :]
                    for tt in range(TT):
                        nc.tensor.matmul(ids_t, s16m[:, tt, :],
                                         m48t[:, tt, :],
                                         start=(tt == 0), stop=(tt == TT - 1))
                        nc.tensor.matmul(w_t, s16m[:, tt, :],
                                         m48c[:, tt, :],
                                         start=(tt == 0), stop=(tt == TT - 1))
                    ids_f = cmp1.tile([16, NC16], F32, name="ids_f", tag="ids_f")
                    nc.vector.tensor_copy(ids_f, ids_t)
                    nc.vector.tensor_copy(W16[k], w_t)
                    bc_ps = zc.tile([P, NC16], F32, name="bc_ps", tag="pz3")
                    nc.tensor.matmul(bc_ps, BI, ids_f, start=True, stop=True)
                    nc.vector.tensor_copy(idx16[k], bc_ps)
                    # gathers for this expert start as soon as idx is ready
                    halves = []
                    for hh, hcap in enumerate(_halves(CAPS[k])):
                        base = hh * ACH
                        xgh = ag.tile([P, KT_H, hcap], BF16, name=f"xg{k}_{hh}",
                                      tag="xg", bufs=2)
                        csl = slice(base // 16, (base + hcap) // 16)
                        nc.gpsimd.dma_gather(
                            xgh, xbf_d.ap(), idx16[k][:, csl],
                            hcap, hcap, H, transpose=True)
                        halves.append(xgh)
                    return halves

                def shared_c_group(hc, s, gi):
                    sw2q = sw2q_t[hc]
                    hsl = slice(hc * 512, (hc + 1) * 512)
                    ssl = slice(s * P, (s + 1) * P)
                    pz = zc.tile([P, 512], F32, name="spz", tag=f"pz{gi % 2}")
                    for ki in range(KT_I):
                        nc.tensor.matmul(pz, ys[:, ki, ssl], sw2q[:, ki, :],
                                         start=(ki == 0), stop=(ki == KT_I - 1))
                    ot = so.tile([P, 512], BF16, name="ot", tag="ot")
                    nc.vector.tensor_copy(ot, pz)
                    nc.scalar.dma_start(out=out_d.ap()[ssl, hsl], in_=ot)

                # PE fill order across the compaction window:
                xg = [None, None]
                xg[0] = compact_mms(0)
                xg[1] = compact_mms(1)
                gi = 0
                for hc in range(4):
                    for s in range(TS // P):
                        shared_c_group(hc, s, gi)
                        gi += 1
                # exports on the scalar HWDGE queue: keeps the Q7 queue
                # free for the critical-path gathers
                for k in range(E_PER_CORE):
                    nc.sync.dma_start(out=ids_d.ap()[k], in_=idx16[k][0:16, :])
                    # W16[q, 8s + r] -> W128[r*16 + q, s]
                    for r in range(8):
                        nc.sync.dma_start(out=W128[k][16 * r:16 * (r + 1), :],
                                          in_=W16[k][:, r::8])

                # ---------------- routed FFN per expert ----------------
                y = [sres.tile([P, KT_I, capc[k]], BF16, name=f"y{k}")
                     for k in range(E_PER_CORE)]

                def pass_a_ch(k, c, h):
                    """pass A quarter: silu(x@w1T)*(x@w3T) for one (c, h)."""
                    hcap = [ACH, capc[k] - ACH][c]
                    w1h, w3h = w1h_t[(k, h)], w3h_t[(k, h)]
                    for m in range(4):
                        mi = h * 4 + m
                        msl = slice(m * P, (m + 1) * P)
                        pg = aps.tile([P, ACH], F32, name="pg",
                                      tag=f"pg{m % 2}")[:, :hcap]
                        pu = aps.tile([P, ACH], F32, name="pu",
                                      tag=f"pu{m % 2}")[:, :hcap]
                        for kt in range(KT_H):
                            nc.tensor.matmul(
                                pg, w1h[:, kt, msl], xg[k][c][:, kt, :hcap],
                                start=(kt == 0), stop=(kt == KT_H - 1))
                        for kt in range(KT_H):
                            nc.tensor.matmul(
                                pu, w3h[:, kt, msl], xg[k][c][:, kt, :hcap],
                                start=(kt == 0), stop=(kt == KT_H - 1))
                        sg = ay.tile([P, ACH], BF16, name="sg",
                                     tag="sg")[:, :hcap]
                        nc.scalar.activation(sg, pg, AF.Silu)
                        csl = slice(c * ACH, c * ACH + hcap)
                        nc.vector.tensor_tensor(y[k][:, mi, csl], sg,
                                                pu, ALU.mult)

                def pass_c_q(k, q, gi0):
                    """pass C for one q block: z = W * (y @ w2T)."""
                    gi = gi0
                    ns_full, rem = divmod(capc[k], P)
                    sdims = [P] * ns_full + ([rem] if rem else [])
                    w2q = w2h_t[(k, q)]
                    hsl = slice(q * 512, (q + 1) * 512)
                    for s, sdim in enumerate(sdims):
                        ssl = slice(s * P, s * P + sdim)
                        pz = zc.tile([P, 512], F32, name="pz",
                                     tag=f"pz{gi % 4}")[:sdim, :]
                        gi += 1
                        for ki in range(KT_I):
                            nc.tensor.matmul(pz, y[k][:, ki, ssl],
                                             w2q[:, ki, :],
                                             start=(ki == 0),
                                             stop=(ki == KT_I - 1))
                        zt = zo.tile([P, 512], BF16, name="zc",
                                     tag="zc")[:sdim, :]
                        nc.vector.tensor_scalar_mul(zt, pz,
                                                    W128[k][:sdim, s:s + 1])
                        nc.sync.dma_start(out=z_d.ap()[k, ssl, hsl], in_=zt)
                    return gi

                # pass A e0, then pass C e0 interleaved with pass A e1 so
                # the per-engine FIFOs (PE, vector, scalar) overlap the
                # two experts instead of head-of-line blocking
                for c in range(2):
                    for h in range(2):
                        pass_a_ch(0, c, h)
                gi = pass_c_q(0, 0, 0)
                pass_a_ch(1, 0, 0)
                gi = pass_c_q(0, 1, gi)
                pass_a_ch(1, 0, 1)
                gi = pass_c_q(0, 2, gi)
                pass_a_ch(1, 1, 0)
                gi = pass_c_q(0, 3, gi)
                pass_a_ch(1, 1, 1)
                gi = 0
                for q in range(4):
                    gi = pass_c_q(1, q, gi)

            es_.close()

    nc.compile()
    return nc


_NC_CACHE = {}


def _get_nc(capc):
    if capc not in _NC_CACHE:
        _NC_CACHE[capc] = build_nc(capc)
    return _NC_CACHE[capc]


def _route_counts(x, gate_w, expert_bias):
    """Host-side routing counts, used ONLY for load-balanced expert->core
    assignment and compute-capacity sizing (sharding decisions); the
    device recomputes routing."""
    logits = x @ gate_w.T
    scores = 1.0 / (1.0 + np.exp(-logits))
    sel = scores + expert_bias[None, :]
    grp = sel.reshape(T, G, E // G)
    t2 = np.sort(grp, -1)[:, :, -2:].sum(-1)
    gidx = np.argsort(t2, -1)[:, -2:]
    gmask = np.zeros((T, G), bool)
    gmask[np.arange(T)[:, None], gidx] = True
    emask = np.repeat(gmask, E // G, axis=1)
    masked = np.where(emask, sel, -np.inf)
    ids = np.argsort(masked, -1)[:, -K_TOP:]
    return np.bincount(ids.ravel(), minlength=E)


def _pc(aT, ncol):
    """[KT*128, C] -> [C//ncol, 128, KT*ncol] partition-contiguous."""
    kt = aT.shape[0] // P
    nch = aT.shape[1] // ncol
    a = aT.reshape(kt, P, nch, ncol)
    return np.ascontiguousarray(
        np.transpose(a, (2, 1, 0, 3)).reshape(nch, P, kt * ncol))


def kernel(hidden_states, gate_w, expert_bias, w1, w3, w2, sw1, sw3, sw2):
    x = np.ascontiguousarray(hidden_states, dtype=np.float32)
    bf = ml_dtypes.bfloat16
    xhi = x.astype(bf)
    xlo = (x - xhi.astype(np.float32)).astype(bf)
    gw = np.ascontiguousarray(gate_w.astype(np.float32))
    ghi = gw.astype(bf)
    glo = (gw - ghi.astype(np.float32)).astype(bf)
    xbf = np.ascontiguousarray(xhi)
    xhiT = np.ascontiguousarray(xhi.T)
    xloT = np.ascontiguousarray(xlo.T)
    bias = expert_bias.astype(np.float32)
    biasb = np.ascontiguousarray(np.broadcast_to(bias[None, :], (P, E)))

    # partition-contiguous streaming layouts
    xhi_r = _pc(xhiT, TCH)                      # [NTOK, P, KT_H*TCH]
    xlo_r = _pc(xloT, TCH)
    gpair = np.concatenate([np.ascontiguousarray(ghi.T),
                            np.zeros((H, E), ghi.dtype),
                            np.ascontiguousarray(glo.T)], axis=1)
    g_r = _pc(np.ascontiguousarray(gpair), 3 * E)[0]
    sw13_r = np.stack([_pc(np.ascontiguousarray(sw1.T.astype(bf)), 256),
                       _pc(np.ascontiguousarray(sw3.T.astype(bf)), 256)])
    sw2_r = _pc(np.ascontiguousarray(sw2.T.astype(bf)), 512)

    # load-balanced assignment: pair i-th largest with i-th smallest
    counts = _route_counts(x.astype(np.float64), gw.astype(np.float64),
                           bias.astype(np.float64))
    order = np.argsort(-counts)
    assign = [(int(order[i]), int(order[E - 1 - i])) for i in range(N_CORES)]
    # compute capacity: actual max per slot + margin, rounded to 8
    cnt0 = max(counts[a] for a, _ in assign)
    cnt1 = max(counts[b] for _, b in assign)
    capc = (min(CAPS[0], max(ACH + 8, -(-(int(cnt0) + 8) // 8) * 8)),
            min(CAPS[1], max(ACH + 8, -(-(int(cnt1) + 8) // 8) * 8)))

    w1tb = np.transpose(w1, (0, 2, 1)).astype(bf)
    w3tb = np.transpose(w3, (0, 2, 1)).astype(bf)
    w2tb = np.transpose(w2, (0, 2, 1)).astype(bf)

    in_maps = []
    for c in range(N_CORES):
        e_hi, e_lo = assign[c]
        esel = np.zeros((P, 2, E), np.float32)
        esel[:, 0, e_hi] = 1.0
        esel[:, 1, e_lo] = 1.0
        pick = [e_hi, e_lo]
        w13_r = np.stack([
            np.stack([
                np.stack([_pc(np.ascontiguousarray(w1tb[e]), 512),
                          _pc(np.ascontiguousarray(w3tb[e]), 512)], axis=1)[h]
                for h in range(2)])
            for e in pick])                     # [2, 2, 2, P, KT_H*512]
        w2_r = np.stack([_pc(np.ascontiguousarray(w2tb[e]), 512)
                         for e in pick])        # [2, 4, P, KT_I*512]
        xbs_r = _pc(np.ascontiguousarray(xhiT[:, TS * c:TS * (c + 1)]), TS)[0]
        in_maps.append({
            "xhi": xhi_r,
            "xlo": xlo_r,
            "xbf": xbf,
            "g": g_r,
            "biasb": biasb,
            "esel": esel,
            "w13": np.ascontiguousarray(w13_r),
            "w2": np.ascontiguousarray(w2_r),
            "sw13": sw13_r,
            "sw2": sw2_r,
            "xbs": xbs_r,
        })

    nc = _get_nc(capc)
    res = run_bass_kernel_spmd(nc, in_maps, list(range(N_CORES)))

    out = np.zeros((T, H), np.float32)
    for c in range(N_CORES):
        r = res.results[c]
        z = np.asarray(r["z"], dtype=np.float32)          # [2, CAPS[0], H]
        ids = np.asarray(r["ids"], dtype=np.int64)        # [2, 16, NC16]
        for k in range(E_PER_CORE):
            slot_ids = ids[k].T.reshape(-1)               # slot i at [i%16, i//16]
            nz = np.nonzero(slot_ids)[0]
            cnt = (nz[-1] + 1) if len(nz) else 0
            cnt = min(cnt, capc[k])
            if cnt:
                out[slot_ids[:cnt]] += z[k, :cnt]
        out[TS * c:TS * (c + 1)] += np.asarray(r["out"], dtype=np.float32)
    kernel.last_result = res
    return out
